# revision 15
# baseline (speedup 1.0000x reference)
"""Trainium2 Bass kernel for nn_DetectionLoss (anchor matching + focal/smooth-L1).

Pure data parallelism: image b runs on core b (B=8). Each core emits 8 partial
scalars; the host combines them (same final reduction as the reference).

Device algorithm (N=65536 anchors, T=32 targets, C=80 classes), v2:
  - Work with the inverted score y = U/I (U = Sa+Sb+eps, I = intersection);
    y is strictly decreasing in IoU, so argmax-IoU = argmin-y and
    iou >= 0.5  <=>  y <= 3 (exactly the reference's threshold algebra).
  - Per t: y computed in ONE scalar_tensor_tensor op ((SA+sbe_t)/I); overlap
    widths via one fused custom DVE op each (relu(min-max)+1e-15, the epsilon
    keeps y finite so no NaN/inf enters the min-reduce); the packed min over t
    carries 12 bits of metadata (7-bit quantized matched-class logit + 5-bit
    target index) in the fp32 mantissa low bits; I-mult and min-accumulate run
    on the otherwise-idle GPSIMD engine.
  - Classification: cls arrives host-transposed as bf16 [C, N]; exp on the
    scalar engine, per-anchor softmax denominator S via PE matmul with a ones
    vector (contraction over the 80 class partitions), landed into anchor-major
    layout by tiny PSUM->SBUF DMAs. ce = log S - x; focal sums via
    tensor_scalar/stt accum_out fused row-sums + PE column-sum.
  - Hard negatives: rank by y ascending among non-positives; k-th threshold by
    bisection on the int32 bit pattern of y (positive floats are bit-ordered),
    host blends the boundary plateau fractionally (exact top-k to ~1e-4).
  - Matched-gt lookup for smooth-L1: 16+16-bit packed gt quantities selected
    per anchor by a custom select-or-accumulate op over the 5-bit target code
    (disjoint one-hot masks => bitwise-OR accumulate).

Host-side prep (sharding/layout only, no loss math): per-image transpose+cast
of cls to bf16 [C, N], gather of the 32 labeled logit columns quantized into
the 12-bit metadata words, and the final 8->1 scalar reduction/blend.
"""

import sys, os

for _p in ("/opt/trn_rl_repo",):
    if _p not in sys.path:
        sys.path.insert(0, _p)

import numpy as np

import concourse.bass as bass
import concourse.bacc as bacc
import concourse.mybir as mybir
from concourse.tile import TileContext
from concourse import bass_utils

F32 = mybir.dt.float32
BF16 = mybir.dt.bfloat16
I32 = mybir.dt.int32
OP = mybir.AluOpType
AF = mybir.ActivationFunctionType

N, C, T = 65536, 80, 32
P, FD = 128, 512  # anchor a = p*FD + f
NCORES = 8
NBIS = 14
CCH = 8192        # anchors per cls-stream chunk
NCH = N // CCH
Y_LO_BITS = 0x40400000   # bits(3.0)
Y_HI_BITS = 0x5D000000   # ~5.8e17, above any finite y (eps=1e-6 keeps y <~ 1.5e17)
POS_OFF = 1e35           # added to y of positives so they rank last

_compiled = None


def _register_dve_op(name, spec):
    from concourse import dve_ops as DOPS
    from concourse.dve_spec import lower
    from concourse.dve_table_gen import DveOpSpec
    if name in DOPS._SUB_OPCODE_FOR_NAME:
        return next(o for o in DOPS.OPS if o.name == name)
    DOPS.OPS.append(DOPS.DveOp(name, spec, False, {}))
    DOPS._SUB_OPCODE_FOR_NAME[name] = DOPS._CUSTOM_DVE_ROW_BASE + len(DOPS.OPS) - 1
    DOPS.CUSTOM_DVE_SPECS[name] = spec
    opc = DOPS.get_dve_sub_opcode(name)
    shas = {}
    for ver in ("v3", "v4"):
        shas[ver] = DveOpSpec(name=name, opcode=opc, uops=lower(spec, ver=ver),
                              rd1_en=DOPS.has_src1(spec)).sha(ver)
    DOPS.OPS[-1] = DOPS.DveOp(name, spec, False, shas)
    return DOPS.OPS[-1]


def _get_ops():
    import numpy as _np
    from concourse.dve_spec import (Spec, Src0, Src1, C0, C1, C2, Zero, relu,
                                    minn, maxx, select, eq, Bin, AluOp)

    def _bits(a):
        return _np.asarray(a, _np.float32).view(_np.int32)

    def _ovlpe_ref(in0, in1, s0, s1, imm2):
        return _np.maximum(_np.minimum(in0, s0) - _np.maximum(in1, s1), 0.0) + imm2

    def _pack_ref(in0, in1, s0, s1):
        m = _bits(s0)
        return ((_bits(in0) & ~m) | _bits(in1)).view(_np.float32)

    def _selacc_ref(in0, in1, s0, s1):
        pick = _np.where(_np.asarray(in0, _np.float32) == _np.float32(s0),
                         _np.broadcast_to(_bits(s1), in0.shape), 0)
        return (_bits(in1) | pick).view(_np.float32)

    # overlap width + tiny epsilon: relu(min(Src0,C0) - max(Src1,C1)) + C2
    ovlpe = _register_dve_op(
        "ANT_DL_OVLPE",
        Spec(body=Bin(AluOp.ADD, relu(minn(Src0, C0) - maxx(Src1, C1)), C2),
             reference=lambda in0, in1, s0, s1, imm2: _ovlpe_ref(in0, in1, s0, s1, imm2)))
    # clear low-12 bits of Src0, OR in Src1 (metadata)
    pack = _register_dve_op(
        "ANT_DL_PACK",
        Spec(body=Bin(AluOp.BITWISE_OR,
                      Bin(AluOp.BITWISE_XOR, Src0,
                          Bin(AluOp.BITWISE_AND, Src0, C0)), Src1),
             reference=lambda in0, in1, s0, s1, imm2: _pack_ref(in0, in1, s0, s1)))
    # select-or-accumulate: Src1 | (Src0 == C0 ? C1 : 0)   (disjoint masks)
    selacc = _register_dve_op(
        "ANT_DL_SELACC",
        Spec(body=Bin(AluOp.BITWISE_OR, Src1, select(eq(Src0, C0), C1, Zero)),
             reference=lambda in0, in1, s0, s1, imm2: _selacc_ref(in0, in1, s0, s1)))
    return ovlpe, pack, selacc


def _build():
    nc = bacc.Bacc("TRN2", target_bir_lowering=False, debug=False,
                   enable_asserts=False, num_devices=NCORES)
    clsT_d = nc.dram_tensor("clsT", [C, N], BF16, kind="ExternalInput")
    meta_d = nc.dram_tensor("meta", [T, P, FD], I32, kind="ExternalInput")
    reg_d = nc.dram_tensor("reg", [P, 4 * FD], F32, kind="ExternalInput")
    anc_d = nc.dram_tensor("anc", [P, 4 * FD], F32, kind="ExternalInput")
    x0_d = nc.dram_tensor("x0", [P, FD], F32, kind="ExternalInput")
    tb_d = nc.dram_tensor("tb", [T, 4], F32, kind="ExternalInput")
    out_d = nc.dram_tensor("out", [1, 16], F32, kind="ExternalOutput")

    with TileContext(nc) as tc:
        _emit(nc, tc, clsT_d, meta_d, reg_d, anc_d, x0_d, tb_d, out_d)
    nc.compile()
    return nc


def _emit(nc, tc, clsT_d, meta_d, reg_d, anc_d, x0_d, tb_d, out_d):
    import contextlib
    ctx = contextlib.ExitStack()
    pool = ctx.enter_context(tc.tile_pool(name="main", bufs=1))
    cpool = ctx.enter_context(tc.tile_pool(name="cls", bufs=2))
    epool = ctx.enter_context(tc.tile_pool(name="exp", bufs=2))
    mpool = ctx.enter_context(tc.tile_pool(name="meta", bufs=2))
    psum = ctx.enter_context(tc.tile_pool(name="ps", bufs=1, space="PSUM"))
    pssum = ctx.enter_context(tc.tile_pool(name="psS", bufs=2, space="PSUM"))
    v, s, g = nc.vector, nc.scalar, nc.gpsimd

    def ts(out, in0, s1, op0, s2=None, op1=None, accum=None, eng=v):
        if accum is not None and op1 is None:
            op1 = OP.add  # accum reduce op rides in op1
        kw = dict(scalar2=s2) if op1 is None else dict(scalar2=s2, op1=op1)
        if accum is not None:
            kw["accum_out"] = accum
        return eng.tensor_scalar(out=out, in0=in0, scalar1=s1, op0=op0, **kw)

    def tt(out, in0, in1, op, eng=v):
        return eng.tensor_tensor(out=out, in0=in0, in1=in1, op=op)

    def stt(out, in0, sc, in1, op0, op1, accum=None, eng=v):
        kw = {} if accum is None else {"accum_out": accum}
        return eng.scalar_tensor_tensor(out=out, in0=in0, scalar=sc, in1=in1,
                                        op0=op0, op1=op1, **kw)

    _ctr = [0]

    def nt(shape, dt=F32):
        _ctr[0] += 1
        return pool.tile(shape, dt, name=f"tl{_ctr[0]}", tag=f"tl{_ctr[0]}")

    OVLPE, PACKOP, SELACC = _get_ops()
    KSTAGE = int(os.environ.get("KSTAGE", "99"))

    def stage_out(n, *vals):
        if KSTAGE != n:
            return False
        SCx = nt([1, 16])
        v.memset(SCx[:, :], 0.0)
        for i, val in enumerate(vals):
            ts(SCx[:, i:i + 1], val, 1.0, OP.bypass)
        nc.sync.dma_start(out_d[:, :], SCx[:, :])
        ctx.close()
        return True

    # ---------------- target prep (tiny) ----------------
    TB = nt([P, 4 * T])
    nc.sync.dma_start(TB[:, :], tb_d.rearrange("t c -> (t c)")[None, :]
                      .broadcast_to([P, 4 * T]))
    TBv = TB[:, :].rearrange("p (t c) -> p t c", c=4)
    tx0, ty0 = TBv[:, :, 0], TBv[:, :, 1]
    tx1, ty1 = TBv[:, :, 2], TBv[:, :, 3]
    WB, HB, SBE = nt([P, T]), nt([P, T]), nt([P, T])
    GCX, GCY, LNW, LNH = nt([P, T]), nt([P, T]), nt([P, T]), nt([P, T])
    tt(WB[:, :], tx1, tx0, OP.subtract)
    tt(HB[:, :], ty1, ty0, OP.subtract)
    tmpT = nt([P, T])
    tt(tmpT[:, :], WB[:, :], HB[:, :], OP.mult)
    ts(SBE[:, :], tmpT[:, :], 1e-6, OP.add)
    stt(GCX[:, :], WB[:, :], 0.5, tx0, OP.mult, OP.add)
    stt(GCY[:, :], HB[:, :], 0.5, ty0, OP.mult, OP.add)
    s.activation(LNW[:, :], WB[:, :], AF.Ln)
    s.activation(LNH[:, :], HB[:, :], AF.Ln)
    # 16+16-bit packed gt coords (x<<16|y), (lnw<<16|lnh)
    QXY, QWH = nt([P, T], I32), nt([P, T], I32)
    q0, q1 = nt([P, T], I32), nt([P, T], I32)
    ts(q0[:, :], GCX[:, :], 32.0, OP.mult)
    ts(q1[:, :], GCY[:, :], 32.0, OP.mult)
    qt = nt([P, T], I32)
    ts(qt[:, :], q0[:, :], 65536, OP.mult)
    tt(QXY[:, :], qt[:, :], q1[:, :], OP.add)
    ts(q0[:, :], LNW[:, :], 4096.0, OP.mult, 8192.0, OP.add)
    ts(q1[:, :], LNH[:, :], 4096.0, OP.mult, 8192.0, OP.add)
    ts(qt[:, :], q0[:, :], 65536, OP.mult)
    tt(QWH[:, :], qt[:, :], q1[:, :], OP.add)
    if stage_out(1, QWH[0:1, 0:1].bitcast(F32)):
        return

    # ---------------- cls stream: exp on ACT, S via PE ----------------
    S_ = nt([P, FD])
    ones80 = nt([C, 1], BF16)
    v.memset(ones80[:, :], 1.0)
    X0 = nt([P, FD])
    nc.sync.dma_start(X0[:, :], x0_d[:, :])
    MCH = CCH // P  # S columns per chunk
    for ch in range(NCH):
        CT = cpool.tile([C, CCH], BF16, name="ct", tag="ct")
        nc.sync.dma_start(CT[:, :], clsT_d[:, ch * CCH:(ch + 1) * CCH])
        s.activation(CT[:, :], CT[:, :], AF.Exp)
        pS = pssum.tile([P, MCH], F32, name="pS", tag="pS")
        for j in range(MCH):
            nc.tensor.matmul(pS[:, j:j + 1], CT[:, j * P:(j + 1) * P],
                             ones80[:, :], start=True, stop=True)
        s.activation(S_[:, ch * MCH:(ch + 1) * MCH], pS[:, :], AF.Copy)
    if stage_out(2, S_[0:1, 0:1], X0[0:1, 0:1]):
        return

    # ---------------- anchor prep ----------------
    RG, AN = nt([P, 4 * FD]), nt([P, 4 * FD])
    nc.sync.dma_start(RG[:, :], reg_d[:, :])
    nc.sync.dma_start(AN[:, :], anc_d[:, :])
    RGv = RG[:, :].rearrange("p (f c) -> p c f", c=4)
    ANv = AN[:, :].rearrange("p (f c) -> p c f", c=4)
    rg0, rg1, rg2, rg3 = (RGv[:, i, :] for i in range(4))
    ax0, ay0, ax1, ay1 = (ANv[:, i, :] for i in range(4))

    AW, AH, ACX, ACY = nt([P, FD]), nt([P, FD]), nt([P, FD]), nt([P, FD])
    tt(AW[:, :], ax1, ax0, OP.subtract)
    tt(AH[:, :], ay1, ay0, OP.subtract)
    stt(ACX[:, :], AW[:, :], 0.5, ax0, OP.mult, OP.add)
    stt(ACY[:, :], AH[:, :], 0.5, ay0, OP.mult, OP.add)

    DW, DH = nt([P, FD]), nt([P, FD])
    s.activation(DW[:, :], rg2, AF.Exp)
    s.activation(DH[:, :], rg3, AF.Exp)
    tt(DW[:, :], DW[:, :], AW[:, :], OP.mult, eng=g)
    tt(DH[:, :], DH[:, :], AH[:, :], OP.mult, eng=g)
    DCX, DCY = nt([P, FD]), nt([P, FD])
    tt(DCX[:, :], rg0, AW[:, :], OP.mult, eng=g)
    tt(DCX[:, :], DCX[:, :], ACX[:, :], OP.add, eng=g)
    tt(DCY[:, :], rg1, AH[:, :], OP.mult, eng=g)
    tt(DCY[:, :], DCY[:, :], ACY[:, :], OP.add, eng=g)
    DX0, DY0, DX1, DY1, SA = (nt([P, FD]) for _ in range(5))
    stt(DX0[:, :], DW[:, :], -0.5, DCX[:, :], OP.mult, OP.add)
    stt(DX1[:, :], DW[:, :], 0.5, DCX[:, :], OP.mult, OP.add)
    stt(DY0[:, :], DH[:, :], -0.5, DCY[:, :], OP.mult, OP.add)
    stt(DY1[:, :], DH[:, :], 0.5, DCY[:, :], OP.mult, OP.add)
    tt(SA[:, :], DW[:, :], DH[:, :], OP.mult)

    # reg-target helpers (bilinear residual form)
    AWE, AHE = nt([P, FD]), nt([P, FD])
    ts(AWE[:, :], AW[:, :], 1e-6, OP.add)
    ts(AHE[:, :], AH[:, :], 1e-6, OP.add)
    RBX, RBY, rsc = nt([P, FD]), nt([P, FD]), nt([P, FD])
    v.reciprocal_approx_accurate(out=RBX[:, :], in_=AWE[:, :], scratch=rsc[:, :])
    v.reciprocal_approx_accurate(out=RBY[:, :], in_=AHE[:, :], scratch=rsc[:, :])
    ALX, ALY, GWl, GHl = nt([P, FD]), nt([P, FD]), nt([P, FD]), nt([P, FD])
    tt(ALX[:, :], ACX[:, :], RBX[:, :], OP.mult, eng=g)
    tt(ALX[:, :], ALX[:, :], rg0, OP.add, eng=g)
    tt(ALY[:, :], ACY[:, :], RBY[:, :], OP.mult, eng=g)
    tt(ALY[:, :], ALY[:, :], rg1, OP.add, eng=g)
    s.activation(GWl[:, :], AWE[:, :], AF.Ln)
    tt(GWl[:, :], GWl[:, :], rg2, OP.add, eng=g)
    s.activation(GHl[:, :], AHE[:, :], AF.Ln)
    tt(GHl[:, :], GHl[:, :], rg3, OP.add, eng=g)
    if stage_out(3, GHl[0:1, 0:1], SA[0:1, 0:1]):
        return

    # ---------------- t-loop: packed min over targets ----------------
    MACC = nt([P, FD])
    v.memset(MACC[:, :].bitcast(I32), Y_HI_BITS + 0x3FFFFF)
    MSKC = nt([P, 1], I32)
    v.memset(MSKC[:, :], 0xFFF)
    ovx, ovy, I_, RI, Y_, YP = (nt([P, FD]) for _ in range(6))
    TPC = 4  # meta rows per streamed chunk
    for t in range(T):
        if t % TPC == 0:
            MT = mpool.tile([P, TPC * FD], I32, name="mt", tag="mt")
            nc.sync.dma_start(
                MT[:, :].rearrange("p (t f) -> p t f", f=FD),
                meta_d[t:t + TPC, :, :].rearrange("t p f -> p t f"))
            MTv = MT[:, :].rearrange("p (t f) -> p t f", f=FD)
        v._custom_dve(OVLPE, out=ovx[:, :], in0=DX1[:, :], in1=DX0[:, :],
                      s0=TB[:, 4 * t + 2:4 * t + 3], s1=TB[:, 4 * t + 0:4 * t + 1],
                      imm2=1e-6)
        v._custom_dve(OVLPE, out=ovy[:, :], in0=DY1[:, :], in1=DY0[:, :],
                      s0=TB[:, 4 * t + 3:4 * t + 4], s1=TB[:, 4 * t + 1:4 * t + 2],
                      imm2=1e-6)
        if KSTAGE == 41:
            continue
        tt(I_[:, :], ovx[:, :], ovy[:, :], OP.mult, eng=g)
        if KSTAGE == 42:
            continue
        v.reciprocal_approx_fast(out=RI[:, :], in_=I_[:, :])
        stt(Y_[:, :], SA[:, :], SBE[:, t:t + 1], RI[:, :], OP.add, OP.mult)
        if KSTAGE == 43:
            continue
        v._custom_dve(PACKOP, out=YP[:, :], in0=Y_[:, :],
                      in1=MTv[:, t % TPC, :].bitcast(F32),
                      s0=MSKC[:, :].bitcast(F32))
        if KSTAGE == 44:
            continue
        tt(MACC[:, :], MACC[:, :], YP[:, :], OP.min)
    if KSTAGE in (41, 42, 43, 44):
        if stage_out(KSTAGE, ovx[0:1, 0:1]):
            return
    if stage_out(4, MACC[0:1, 0:1]):
        return

    # ---------------- selection scalars ----------------
    SC = nt([1, 16])
    v.memset(SC[:, :], 0.0)
    ones_col = nt([P, 1])
    v.memset(ones_col[:, :], 1.0)
    ones_row = nt([1, P])
    v.memset(ones_row[:, :], 1.0)
    acc_np, acc_cnt, acc_sum, acc_cp, acc_sl = (nt([P, 1]) for _ in range(5))

    def psum_scalar(src_col, dst):  # [P,1] -> [1,1]
        pt = psum.tile([1, 1], F32, name="pss", tag="pss")
        nc.tensor.matmul(pt[:, :], src_col, ones_col[:, :], start=True, stop=True)
        ts(dst, pt[:, :], 1.0, OP.mult)

    def bcast_col(src_sc):  # [1,1] -> [P,1]
        bc = psum.tile([P, 1], F32, name="bcc", tag="bcc")
        nc.tensor.matmul(bc[:, :], ones_row[:, :], src_sc, start=True, stop=True)
        bcs = nt([P, 1])
        s.activation(bcs[:, :], bc[:, :], AF.Copy)
        return bcs

    mi = MACC[:, :].bitcast(I32)
    POSM, NEGY = AW, AH  # reuse dead prep tiles
    ts(POSM[:, :], MACC[:, :], 3.0, OP.is_le, accum=acc_np[:, :])
    npos_t = SC[:, 0:1]
    psum_scalar(acc_np[:, :], npos_t)
    stt(NEGY[:, :], POSM[:, :], POS_OFF, MACC[:, :], OP.mult, OP.add)

    # k = min(4*npos, N-npos)
    k_t = SC[:, 1:2]
    kA, kB = nt([1, 1]), nt([1, 1])
    ts(kA[:, :], npos_t, 4.0, OP.mult)
    ts(kB[:, :], npos_t, -1.0, OP.mult, float(N), OP.add)
    tt(k_t, kA[:, :], kB[:, :], OP.min)

    # decode metadata from MACC
    tcf, XL = ACX, ACY  # reuse
    xqi = nt([P, FD], I32)
    ts(xqi[:, :], mi, 31, OP.bitwise_and)
    ts(tcf[:, :], xqi[:, :], 1.0, OP.mult)          # tcode as f32
    ts(xqi[:, :], mi, 5, OP.arith_shift_right, 0x7F, OP.bitwise_and)
    ts(XL[:, :], xqi[:, :], 0.125, OP.mult, -8.0625, OP.add)
    if stage_out(5, npos_t, k_t, XL[0:1, 0:1], tcf[0:1, 0:1]):
        return

    # ---------------- fl_neg (dense) + cls_pos ----------------
    LSE, CE0 = DW, DH  # reuse
    s.activation(LSE[:, :], S_[:, :], AF.Ln)
    tt(CE0[:, :], LSE[:, :], X0[:, :], OP.subtract, eng=g)
    P0, u_, FLN = DCY, ovx, ovy
    s.activation(P0[:, :], CE0[:, :], AF.Exp, scale=-1.0)
    ts(u_[:, :], P0[:, :], -1.0, OP.mult, 1.0, OP.add)
    tt(FLN[:, :], u_[:, :], u_[:, :], OP.mult, eng=g)
    stt(FLN[:, :], FLN[:, :], 0.1, u_[:, :], OP.mult, OP.mult)
    tt(FLN[:, :], FLN[:, :], CE0[:, :], OP.mult, eng=g)

    CEP, PP = I_, Y_
    tt(CEP[:, :], LSE[:, :], XL[:, :], OP.subtract, eng=g)
    s.activation(PP[:, :], CEP[:, :], AF.Exp, scale=-1.0)
    ts(u_[:, :], PP[:, :], -1.0, OP.mult, 1.0, OP.add)
    tt(PP[:, :], u_[:, :], u_[:, :], OP.mult, eng=g)
    stt(PP[:, :], PP[:, :], 0.25, CEP[:, :], OP.mult, OP.mult)
    stt(YP[:, :], PP[:, :], 1.0, POSM[:, :], OP.mult, OP.mult,
        accum=acc_cp[:, :])
    psum_scalar(acc_cp[:, :], SC[:, 6:7])  # cls_pos
    if stage_out(6, SC[:, 6:7], FLN[0:1, 0:1]):
        return

    # ---------------- gt lookup (select-or-accumulate) ----------------
    AXYf = DX0[:, :]
    AWHf = DX1[:, :]
    AXY = AXYf.bitcast(I32)
    AWH = AWHf.bitcast(I32)
    v.memset(AXY, 0)
    v.memset(AWH, 0)
    tcfF = tcf[:, :]
    for t in range(T):
        v._custom_dve(SELACC, out=AXYf, in0=tcfF, in1=AXYf,
                      s0=float(t), s1=QXY[:, t:t + 1].bitcast(F32))
        v._custom_dve(SELACC, out=AWHf, in0=tcfF, in1=AWHf,
                      s0=float(t), s1=QWH[:, t:t + 1].bitcast(F32))

    if stage_out(7, AXYf[0:1, 0:1], AWHf[0:1, 0:1]):
        return
    # ---------------- bisection for the k-th negative threshold ----------
    lo, hi = nt([1, 1], I32), nt([1, 1], I32)
    v.memset(lo[:, :], Y_LO_BITS)
    v.memset(hi[:, :], Y_HI_BITS)
    mid, d_s = nt([1, 1], I32), nt([1, 1], I32)
    sel_i, nsel_i = nt([1, 1], I32), nt([1, 1], I32)
    sel_f = nt([1, 1])
    tmp1 = nt([1, 1], I32)
    geM = SA  # reuse
    for it in range(NBIS):
        tt(d_s[:, :], hi[:, :], lo[:, :], OP.subtract)
        ts(d_s[:, :], d_s[:, :], 1, OP.arith_shift_right)
        tt(mid[:, :], lo[:, :], d_s[:, :], OP.add)
        bc = psum.tile([P, 1], F32, name="bcc", tag="bcc")
        nc.tensor.matmul(bc[:, :], ones_row[:, :], mid[:, :].bitcast(F32),
                         start=True, stop=True)
        ts(geM[:, :], NEGY[:, :], bc[:, 0:1], OP.is_le, accum=acc_cnt[:, :])
        pt = psum.tile([1, 1], F32, name="psb", tag="psb")
        nc.tensor.matmul(pt[:, :], acc_cnt[:, :], ones_col[:, :],
                         start=True, stop=True)
        tt(sel_f[:, :], pt[:, :], k_t, OP.is_ge)
        ts(sel_i[:, :], sel_f[:, :], 1, OP.mult)
        ts(nsel_i[:, :], sel_i[:, :], -1, OP.mult, 1, OP.add)  # 1-sel
        # sel ? hi=mid : lo=mid
        tt(d_s[:, :], mid[:, :], hi[:, :], OP.subtract)
        tt(d_s[:, :], d_s[:, :], sel_i[:, :], OP.mult)
        tt(hi[:, :], hi[:, :], d_s[:, :], OP.add)
        tt(tmp1[:, :], mid[:, :], lo[:, :], OP.subtract)
        tt(tmp1[:, :], tmp1[:, :], nsel_i[:, :], OP.mult)
        tt(lo[:, :], lo[:, :], tmp1[:, :], OP.add)

    def masked_sums(thr_i, cnt_dst, sum_dst):
        thc = bcast_col(thr_i[:, :].bitcast(F32))
        ts(geM[:, :], NEGY[:, :], thc[:, 0:1], OP.is_le, accum=acc_cnt[:, :])
        psum_scalar(acc_cnt[:, :], cnt_dst)
        stt(YP[:, :], FLN[:, :], 1.0, geM[:, :], OP.mult, OP.mult,
            accum=acc_sum[:, :])
        psum_scalar(acc_sum[:, :], sum_dst)

    masked_sums(lo, SC[:, 2:3], SC[:, 3:4])   # c_lo, s_lo  (count < k)
    masked_sums(hi, SC[:, 4:5], SC[:, 5:6])   # c_hi, s_hi  (count >= k)
    if stage_out(8, SC[:, 2:3], SC[:, 3:4], SC[:, 4:5], SC[:, 5:6]):
        return

    # ---------------- unpack gt + smooth-L1 ----------------
    Gx, Gy, Gw, Gh = (ANv[:, i, :] for i in range(4))  # reuse AN storage
    gq = xqi
    ts(gq[:, :], AXY, 16, OP.arith_shift_right)
    ts(Gx, gq[:, :], 0.03125, OP.mult)
    ts(gq[:, :], AXY, 0xFFFF, OP.bitwise_and)
    ts(Gy, gq[:, :], 0.03125, OP.mult)
    ts(gq[:, :], AWH, 16, OP.arith_shift_right)
    ts(Gw, gq[:, :], 0.000244140625, OP.mult, -2.0, OP.add)
    ts(gq[:, :], AWH, 0xFFFF, OP.bitwise_and)
    ts(Gh, gq[:, :], 0.000244140625, OP.mult, -2.0, OP.add)

    SL = CE0  # reuse
    R = P0
    v.memset(SL[:, :], 0.0)
    sa_, sq_, sl1_, slt = LSE, DCX, u_, FLN  # careful: FLN dead after sums
    # NOTE: FLN used in masked_sums above; slt reuse is safe because sl1()
    # runs after masked_sums in program order on each engine queue.
    sl1tmp = nt([P, FD])

    def sl1(resid):
        ts(sa_[:, :].bitcast(I32), resid.bitcast(I32), 0x7FFFFFFF, OP.bitwise_and)
        tt(sq_[:, :], resid, resid, OP.mult, eng=g)
        ts(sl1_[:, :], sa_[:, :], -0.5, OP.add)
        ts(slt[:, :], sa_[:, :], 1.0, OP.is_lt)
        stt(sl1tmp[:, :], sq_[:, :], 0.5, sl1_[:, :], OP.mult, OP.subtract)
        tt(sl1tmp[:, :], sl1tmp[:, :], slt[:, :], OP.mult, eng=g)
        tt(sl1tmp[:, :], sl1tmp[:, :], sl1_[:, :], OP.add, eng=g)
        tt(SL[:, :], SL[:, :], sl1tmp[:, :], OP.add, eng=g)

    tt(R[:, :], Gx, RBX[:, :], OP.mult, eng=g)
    tt(R[:, :], ALX[:, :], R[:, :], OP.subtract)
    sl1(R[:, :])
    tt(R[:, :], Gy, RBY[:, :], OP.mult, eng=g)
    tt(R[:, :], ALY[:, :], R[:, :], OP.subtract)
    sl1(R[:, :])
    tt(R[:, :], GWl[:, :], Gw, OP.subtract)
    sl1(R[:, :])
    tt(R[:, :], GHl[:, :], Gh, OP.subtract)
    sl1(R[:, :])
    stt(YP[:, :], SL[:, :], 1.0, POSM[:, :], OP.mult, OP.mult,
        accum=acc_sl[:, :])
    psum_scalar(acc_sl[:, :], SC[:, 7:8])  # sl1 sum
    nc.sync.dma_start(out_d[:, :], SC[:, :])
    ctx.close()


def _make_in_maps(cls_output, reg_output, anchors, target_boxes, target_labels):
    B = cls_output.shape[0]
    import ml_dtypes
    bf16 = ml_dtypes.bfloat16
    in_maps = []
    tcode = np.arange(T, dtype=np.int32)[:, None, None]
    # anchor-minor layout: a = f*128 + p  ->  [p, f]
    anc_pf = np.ascontiguousarray(
        np.asarray(anchors, np.float32).reshape(FD, P, 4).swapaxes(0, 1)
        .reshape(P, 4 * FD))
    for b in range(B):
        cls_b = np.asarray(cls_output[b], dtype=np.float32)
        labels_b = np.asarray(target_labels[b]).astype(np.int64)
        # 7-bit quantized gathered logits + 5-bit target index, [T, P, FD]
        xl = cls_b[:, labels_b].T.reshape(T, FD, P).swapaxes(1, 2)
        xq = np.clip(np.floor(xl * 8.0 + 64.5), 0, 127).astype(np.int32)
        meta = (xq << 5) | tcode
        in_maps.append({
            "clsT": np.ascontiguousarray(cls_b.T).astype(bf16),
            "meta": np.ascontiguousarray(meta),
            "reg": np.ascontiguousarray(
                np.asarray(reg_output[b], np.float32).reshape(FD, P, 4)
                .swapaxes(0, 1).reshape(P, 4 * FD)),
            "anc": anc_pf,
            "x0": np.ascontiguousarray(cls_b[:, 0].reshape(FD, P).T),
            "tb": np.ascontiguousarray(target_boxes[b], dtype=np.float32),
        })
    return in_maps


def kernel(cls_output, reg_output, anchors, target_boxes, target_labels):
    global _compiled
    if _compiled is None:
        _compiled = _build()
    nc = _compiled
    B = cls_output.shape[0]
    in_maps = _make_in_maps(cls_output, reg_output, anchors, target_boxes,
                            target_labels)
    res = bass_utils.run_bass_kernel_spmd(nc, in_maps, core_ids=list(range(B)))

    cls_l = np.zeros(B, np.float32)
    reg_l = np.zeros(B, np.float32)
    npos_a = np.zeros(B, np.int64)
    for b in range(B):
        sc = res.results[b]["out"][0]
        npos, k = float(sc[0]), float(sc[1])
        c_lo, s_lo, c_hi, s_hi = float(sc[2]), float(sc[3]), float(sc[4]), float(sc[5])
        cls_pos, sl1s = float(sc[6]), float(sc[7])
        # fractional blend on the bisection plateau: exactly k negatives
        if c_hi > c_lo:
            frac = (k - c_lo) / (c_hi - c_lo)
        else:
            frac = 0.0
        cls_neg = s_lo + frac * (s_hi - s_lo)
        total = max(npos + k, 1.0)
        cls_l[b] = np.float32((cls_pos + cls_neg) / total)
        reg_l[b] = np.float32(sl1s / (npos + 1e-6))
        npos_a[b] = int(round(npos))

    total_pos = np.int32(npos_a.sum())
    cls_final = np.float32(cls_l.mean())
    reg_final = np.float32(reg_l.mean()) if total_pos > 0 else np.float32(0.0)
    reg_weight = np.float32(min(1.0, float(total_pos) / (100.0 * B)))
    total_loss = np.float32(cls_final + reg_weight * 1.0 * reg_final)
    return (total_loss, cls_final, reg_final, np.int32(total_pos))


# revision 19
# speedup vs baseline: 1.2581x; 1.2581x over previous
"""Trainium2 Bass kernel for nn_DetectionLoss (anchor matching + focal/smooth-L1).

Pure data parallelism: image b runs on core b (B=8). Each core emits 8 partial
scalars; the host combines them (same final reduction as the reference).

Device algorithm (N=65536 anchors, T=32 targets, C=80 classes), v2:
  - Work with the inverted score y = U/I (U = Sa+Sb+eps, I = intersection);
    y is strictly decreasing in IoU, so argmax-IoU = argmin-y and
    iou >= 0.5  <=>  y <= 3 (exactly the reference's threshold algebra).
  - Per t: y computed in ONE scalar_tensor_tensor op ((SA+sbe_t)/I); overlap
    widths via one fused custom DVE op each (relu(min-max)+1e-15, the epsilon
    keeps y finite so no NaN/inf enters the min-reduce); the packed min over t
    carries 12 bits of metadata (7-bit quantized matched-class logit + 5-bit
    target index) in the fp32 mantissa low bits; I-mult and min-accumulate run
    on the otherwise-idle GPSIMD engine.
  - Classification: cls arrives host-transposed as bf16 [C, N]; exp on the
    scalar engine, per-anchor softmax denominator S via PE matmul with a ones
    vector (contraction over the 80 class partitions), landed into anchor-major
    layout by tiny PSUM->SBUF DMAs. ce = log S - x; focal sums via
    tensor_scalar/stt accum_out fused row-sums + PE column-sum.
  - Hard negatives: rank by y ascending among non-positives; k-th threshold by
    bisection on the int32 bit pattern of y (positive floats are bit-ordered),
    host blends the boundary plateau fractionally (exact top-k to ~1e-4).
  - Matched-gt lookup for smooth-L1: 16+16-bit packed gt quantities selected
    per anchor by a custom select-or-accumulate op over the 5-bit target code
    (disjoint one-hot masks => bitwise-OR accumulate).

Host-side prep (sharding/layout only, no loss math): per-image transpose+cast
of cls to bf16 [C, N], gather of the 32 labeled logit columns quantized into
the 12-bit metadata words, and the final 8->1 scalar reduction/blend.
"""

import sys, os

for _p in ("/opt/trn_rl_repo",):
    if _p not in sys.path:
        sys.path.insert(0, _p)

import numpy as np

import concourse.bass as bass
import concourse.bacc as bacc
import concourse.mybir as mybir
from concourse.tile import TileContext
from concourse import bass_utils

F32 = mybir.dt.float32
BF16 = mybir.dt.bfloat16
I32 = mybir.dt.int32
OP = mybir.AluOpType
AF = mybir.ActivationFunctionType

N, C, T = 65536, 80, 32
P, FD = 128, 512  # anchor a = p*FD + f
NCORES = 8
NBIS = 14
CCH = 4096        # anchors per cls-stream chunk
NCH = N // CCH
Y_LO_BITS = 0x40400000   # bits(3.0)
Y_HI_BITS = 0x5D000000   # ~5.8e17, above any finite y (eps=1e-6 keeps y <~ 1.5e17)
POS_OFF = 1e35           # added to y of positives so they rank last

_compiled = None


def _register_dve_op(name, spec):
    from concourse import dve_ops as DOPS
    from concourse.dve_spec import lower
    from concourse.dve_table_gen import DveOpSpec
    if name in DOPS._SUB_OPCODE_FOR_NAME:
        return next(o for o in DOPS.OPS if o.name == name)
    DOPS.OPS.append(DOPS.DveOp(name, spec, False, {}))
    DOPS._SUB_OPCODE_FOR_NAME[name] = DOPS._CUSTOM_DVE_ROW_BASE + len(DOPS.OPS) - 1
    DOPS.CUSTOM_DVE_SPECS[name] = spec
    opc = DOPS.get_dve_sub_opcode(name)
    shas = {}
    for ver in ("v3", "v4"):
        shas[ver] = DveOpSpec(name=name, opcode=opc, uops=lower(spec, ver=ver),
                              rd1_en=DOPS.has_src1(spec)).sha(ver)
    DOPS.OPS[-1] = DOPS.DveOp(name, spec, False, shas)
    return DOPS.OPS[-1]


def _get_ops():
    import numpy as _np
    from concourse.dve_spec import (Spec, Src0, Src1, C0, C1, C2, Zero, relu,
                                    minn, maxx, select, eq, Bin, AluOp)

    def _bits(a):
        return _np.asarray(a, _np.float32).view(_np.int32)

    def _ovlpe_ref(in0, in1, s0, s1, imm2):
        return _np.maximum(_np.minimum(in0, s0) - _np.maximum(in1, s1), 0.0) + imm2

    def _pack_ref(in0, in1, s0, s1):
        m = _bits(s0)
        return ((_bits(in0) & ~m) | _bits(in1)).view(_np.float32)

    def _selacc_ref(in0, in1, s0, s1):
        pick = _np.where(_np.asarray(in0, _np.float32) == _np.float32(s0),
                         _np.broadcast_to(_bits(s1), in0.shape), 0)
        return (_bits(in1) | pick).view(_np.float32)

    # overlap width + tiny epsilon: relu(min(Src0,C0) - max(Src1,C1)) + C2
    ovlpe = _register_dve_op(
        "ANT_DL_OVLPE",
        Spec(body=Bin(AluOp.ADD, relu(minn(Src0, C0) - maxx(Src1, C1)), C2),
             reference=lambda in0, in1, s0, s1, imm2: _ovlpe_ref(in0, in1, s0, s1, imm2)))
    # clear low-12 bits of Src0, OR in Src1 (metadata)
    pack = _register_dve_op(
        "ANT_DL_PACK",
        Spec(body=Bin(AluOp.BITWISE_OR,
                      Bin(AluOp.BITWISE_XOR, Src0,
                          Bin(AluOp.BITWISE_AND, Src0, C0)), Src1),
             reference=lambda in0, in1, s0, s1, imm2: _pack_ref(in0, in1, s0, s1)))
    # select-or-accumulate: Src1 | (Src0 == C0 ? C1 : 0)   (disjoint masks)
    selacc = _register_dve_op(
        "ANT_DL_SELACC",
        Spec(body=Bin(AluOp.BITWISE_OR, Src1, select(eq(Src0, C0), C1, Zero)),
             reference=lambda in0, in1, s0, s1, imm2: _selacc_ref(in0, in1, s0, s1)))
    return ovlpe, pack, selacc


def _build():
    nc = bacc.Bacc("TRN2", target_bir_lowering=False, debug=False,
                   enable_asserts=False, num_devices=NCORES)
    clsT_d = nc.dram_tensor("clsT", [C, N], BF16, kind="ExternalInput")
    meta_d = nc.dram_tensor("meta", [T, P, FD], I32, kind="ExternalInput")
    reg_d = nc.dram_tensor("reg", [P, 4 * FD], F32, kind="ExternalInput")
    anc_d = nc.dram_tensor("anc", [P, 4 * FD], F32, kind="ExternalInput")
    x0_d = nc.dram_tensor("x0", [P, FD], F32, kind="ExternalInput")
    tb_d = nc.dram_tensor("tb", [T, 4], F32, kind="ExternalInput")
    out_d = nc.dram_tensor("out", [1, 16], F32, kind="ExternalOutput")

    with TileContext(nc) as tc:
        _emit(nc, tc, clsT_d, meta_d, reg_d, anc_d, x0_d, tb_d, out_d)
    nc.compile()
    return nc


def _emit(nc, tc, clsT_d, meta_d, reg_d, anc_d, x0_d, tb_d, out_d):
    import contextlib
    ctx = contextlib.ExitStack()
    pool = ctx.enter_context(tc.tile_pool(name="main", bufs=1))
    cpool = ctx.enter_context(tc.tile_pool(name="cls", bufs=2))
    mpool = ctx.enter_context(tc.tile_pool(name="meta", bufs=2))
    psum = ctx.enter_context(tc.tile_pool(name="ps", bufs=1, space="PSUM"))
    pssum = ctx.enter_context(tc.tile_pool(name="psS", bufs=2, space="PSUM"))
    v, s, g = nc.vector, nc.scalar, nc.gpsimd

    def ts(out, in0, s1, op0, s2=None, op1=None, accum=None, eng=v):
        if accum is not None and op1 is None:
            op1 = OP.add  # accum reduce op rides in op1
        kw = dict(scalar2=s2) if op1 is None else dict(scalar2=s2, op1=op1)
        if accum is not None:
            kw["accum_out"] = accum
        return eng.tensor_scalar(out=out, in0=in0, scalar1=s1, op0=op0, **kw)

    def tt(out, in0, in1, op, eng=v):
        return eng.tensor_tensor(out=out, in0=in0, in1=in1, op=op)

    def stt(out, in0, sc, in1, op0, op1, accum=None, eng=v):
        kw = {} if accum is None else {"accum_out": accum}
        return eng.scalar_tensor_tensor(out=out, in0=in0, scalar=sc, in1=in1,
                                        op0=op0, op1=op1, **kw)

    _ctr = [0]

    def nt(shape, dt=F32):
        _ctr[0] += 1
        return pool.tile(shape, dt, name=f"tl{_ctr[0]}", tag=f"tl{_ctr[0]}")

    OVLPE, PACKOP, SELACC = _get_ops()

    # ---------------- input DMAs (priority order: t-loop deps first) -------
    TB = nt([P, 4 * T])
    nc.sync.dma_start(TB[:, :], tb_d.rearrange("t c -> (t c)")[None, :]
                      .broadcast_to([P, 4 * T]))
    RG, AN = nt([P, 4 * FD]), nt([P, 4 * FD])
    nc.sync.dma_start(AN[:, :], anc_d[:, :])
    nc.sync.dma_start(RG[:, :], reg_d[:, :])
    X0 = nt([P, FD])
    nc.sync.dma_start(X0[:, :], x0_d[:, :])

    # ---------------- target prep (tiny) ----------------
    TBv = TB[:, :].rearrange("p (t c) -> p t c", c=4)
    tx0, ty0 = TBv[:, :, 0], TBv[:, :, 1]
    tx1, ty1 = TBv[:, :, 2], TBv[:, :, 3]
    WB, HB, SBE = nt([P, T]), nt([P, T]), nt([P, T])
    GCX, GCY, LNW, LNH = nt([P, T]), nt([P, T]), nt([P, T]), nt([P, T])
    tt(WB[:, :], tx1, tx0, OP.subtract)
    tt(HB[:, :], ty1, ty0, OP.subtract)
    tmpT = nt([P, T])
    tt(tmpT[:, :], WB[:, :], HB[:, :], OP.mult)
    ts(SBE[:, :], tmpT[:, :], 1e-6, OP.add)
    stt(GCX[:, :], WB[:, :], 0.5, tx0, OP.mult, OP.add)
    stt(GCY[:, :], HB[:, :], 0.5, ty0, OP.mult, OP.add)
    s.activation(LNW[:, :], WB[:, :], AF.Ln)
    s.activation(LNH[:, :], HB[:, :], AF.Ln)
    # 16+16-bit packed gt coords (x<<16|y), (lnw<<16|lnh)
    QXY, QWH = nt([P, T], I32), nt([P, T], I32)
    q0, q1 = nt([P, T], I32), nt([P, T], I32)
    ts(q0[:, :], GCX[:, :], 32.0, OP.mult)
    ts(q1[:, :], GCY[:, :], 32.0, OP.mult)
    qt = nt([P, T], I32)
    ts(qt[:, :], q0[:, :], 65536, OP.mult)
    tt(QXY[:, :], qt[:, :], q1[:, :], OP.add)
    ts(q0[:, :], LNW[:, :], 4096.0, OP.mult, 8192.0, OP.add)
    ts(q1[:, :], LNH[:, :], 4096.0, OP.mult, 8192.0, OP.add)
    ts(qt[:, :], q0[:, :], 65536, OP.mult)
    tt(QWH[:, :], qt[:, :], q1[:, :], OP.add)

    # constants / accumulators early (fills the input-DMA wait)
    SC = nt([1, 16])
    v.memset(SC[:, :], 0.0)
    ones_col = nt([P, 1])
    v.memset(ones_col[:, :], 1.0)
    ones_row = nt([1, P])
    v.memset(ones_row[:, :], 1.0)
    acc_np, acc_cnt, acc_sum, acc_cp, acc_sl = (nt([P, 1]) for _ in range(5))
    MACC = nt([P, FD])
    v.memset(MACC[:, :].bitcast(I32), Y_HI_BITS + 0x3FFFFF)
    MSKC = nt([P, 1], I32)
    v.memset(MSKC[:, :], 0xFFF)
    lo, hi = nt([1, 1], I32), nt([1, 1], I32)
    v.memset(lo[:, :], Y_LO_BITS)
    v.memset(hi[:, :], Y_HI_BITS)

    # ---------------- anchor prep ----------------
    RGv = RG[:, :].rearrange("p (f c) -> p c f", c=4)
    ANv = AN[:, :].rearrange("p (f c) -> p c f", c=4)
    rg0, rg1, rg2, rg3 = (RGv[:, i, :] for i in range(4))
    ax0, ay0, ax1, ay1 = (ANv[:, i, :] for i in range(4))

    AW, AH, ACX, ACY = nt([P, FD]), nt([P, FD]), nt([P, FD]), nt([P, FD])
    tt(AW[:, :], ax1, ax0, OP.subtract)
    tt(AH[:, :], ay1, ay0, OP.subtract)
    stt(ACX[:, :], AW[:, :], 0.5, ax0, OP.mult, OP.add)
    stt(ACY[:, :], AH[:, :], 0.5, ay0, OP.mult, OP.add)

    DW, DH = nt([P, FD]), nt([P, FD])
    s.activation(DW[:, :], rg2, AF.Exp)
    s.activation(DH[:, :], rg3, AF.Exp)
    tt(DW[:, :], DW[:, :], AW[:, :], OP.mult, eng=g)
    tt(DH[:, :], DH[:, :], AH[:, :], OP.mult, eng=g)
    DCX, DCY = nt([P, FD]), nt([P, FD])
    tt(DCX[:, :], rg0, AW[:, :], OP.mult, eng=g)
    tt(DCX[:, :], DCX[:, :], ACX[:, :], OP.add, eng=g)
    tt(DCY[:, :], rg1, AH[:, :], OP.mult, eng=g)
    tt(DCY[:, :], DCY[:, :], ACY[:, :], OP.add, eng=g)
    DX0, DY0, DX1, DY1, SA = (nt([P, FD]) for _ in range(5))
    stt(DX0[:, :], DW[:, :], -0.5, DCX[:, :], OP.mult, OP.add)
    stt(DX1[:, :], DW[:, :], 0.5, DCX[:, :], OP.mult, OP.add)
    stt(DY0[:, :], DH[:, :], -0.5, DCY[:, :], OP.mult, OP.add)
    stt(DY1[:, :], DH[:, :], 0.5, DCY[:, :], OP.mult, OP.add)
    tt(SA[:, :], DW[:, :], DH[:, :], OP.mult)

    # reg-target helpers (bilinear residual form)
    AWE, AHE = nt([P, FD]), nt([P, FD])
    ts(AWE[:, :], AW[:, :], 1e-6, OP.add)
    ts(AHE[:, :], AH[:, :], 1e-6, OP.add)
    RBX, RBY, rsc = nt([P, FD]), nt([P, FD]), nt([P, FD])
    v.reciprocal_approx_accurate(out=RBX[:, :], in_=AWE[:, :], scratch=rsc[:, :])
    v.reciprocal_approx_accurate(out=RBY[:, :], in_=AHE[:, :], scratch=rsc[:, :])
    ALX, ALY, GWl, GHl = nt([P, FD]), nt([P, FD]), nt([P, FD]), nt([P, FD])
    s.activation(GWl[:, :], AWE[:, :], AF.Ln)
    s.activation(GHl[:, :], AHE[:, :], AF.Ln)
    prep_late = [
        lambda: tt(ALX[:, :], ACX[:, :], RBX[:, :], OP.mult, eng=g),
        lambda: tt(ALX[:, :], ALX[:, :], rg0, OP.add, eng=g),
        lambda: tt(ALY[:, :], ACY[:, :], RBY[:, :], OP.mult, eng=g),
        lambda: tt(ALY[:, :], ALY[:, :], rg1, OP.add, eng=g),
        lambda: tt(GWl[:, :], GWl[:, :], rg2, OP.add, eng=g),
        lambda: tt(GHl[:, :], GHl[:, :], rg3, OP.add, eng=g),
    ]

    # ---------------- cls stream pieces (embedded in t-loop) --------------
    S_ = nt([P, FD])
    ones80 = nt([C, 1], BF16)
    v.memset(ones80[:, :], 1.0)
    MCH = CCH // P  # S columns per chunk

    def emit_cls_chunk(ch):
        CT = cpool.tile([C, CCH], BF16, name="ct", tag="ct")
        nc.sync.dma_start(CT[:, :], clsT_d[:, ch * CCH:(ch + 1) * CCH])
        s.activation(CT[:, :], CT[:, :], AF.Exp)
        pS = pssum.tile([P, MCH], F32, name="pS", tag="pS")
        for j in range(MCH):
            nc.tensor.matmul(pS[:, j:j + 1], CT[:, j * P:(j + 1) * P],
                             ones80[:, :], start=True, stop=True)
        s.activation(S_[:, ch * MCH:(ch + 1) * MCH], pS[:, :], AF.Copy)

    # ---------------- t-loop: packed min over targets (sw-pipelined) ------
    TPC = 4  # meta rows per streamed chunk
    NB = 2
    bufs = [dict(ovx=nt([P, FD]), ovy=nt([P, FD]), I=nt([P, FD]),
                 RI=nt([P, FD]), Y=nt([P, FD]), YP=nt([P, FD]),
                 U=nt([P, FD])) for _ in range(NB)]
    mtv = [None, None]

    def fetch_meta(c):
        if c >= T // TPC:
            return
        MT = mpool.tile([P, TPC * FD], I32, name="mt", tag="mt")
        s.dma_start(MT[:, :].rearrange("p (t f) -> p t f", f=FD),
                    meta_d[c * TPC:(c + 1) * TPC, :, :]
                    .rearrange("t p f -> p t f"))
        mtv[c % 2] = MT[:, :].rearrange("p (t f) -> p t f", f=FD)

    def emit_front(t):
        b = bufs[t % NB]
        if t % TPC == 0:
            fetch_meta(t // TPC + 1)  # prefetch next chunk
        v._custom_dve(OVLPE, out=b["ovx"][:, :], in0=DX1[:, :], in1=DX0[:, :],
                      s0=TB[:, 4 * t + 2:4 * t + 3],
                      s1=TB[:, 4 * t + 0:4 * t + 1], imm2=1e-6)
        v._custom_dve(OVLPE, out=b["ovy"][:, :], in0=DY1[:, :], in1=DY0[:, :],
                      s0=TB[:, 4 * t + 3:4 * t + 4],
                      s1=TB[:, 4 * t + 1:4 * t + 2], imm2=1e-6)
        s.activation(b["U"][:, :], SA[:, :], AF.Identity, bias=SBE[:, t:t + 1])
        tt(b["I"][:, :], b["ovx"][:, :], b["ovy"][:, :], OP.mult, eng=g)

    def emit_back(t):
        b = bufs[t % NB]
        v.reciprocal_approx_fast(out=b["RI"][:, :], in_=b["I"][:, :])
        tt(b["Y"][:, :], b["U"][:, :], b["RI"][:, :], OP.mult, eng=g)
        v._custom_dve(PACKOP, out=b["YP"][:, :], in0=b["Y"][:, :],
                      in1=mtv[(t // TPC) % 2][:, t % TPC, :].bitcast(F32),
                      s0=MSKC[:, :].bitcast(F32))
        tt(MACC[:, :], MACC[:, :], b["YP"][:, :], OP.min)

    fetch_meta(0)
    emit_front(0)
    for t in range(1, T):
        emit_front(t)
        if (t - 1) % 2 == 0:
            emit_cls_chunk((t - 1) // 2)
        if prep_late:
            prep_late.pop(0)()
        emit_back(t - 1)
    emit_cls_chunk(15)
    emit_back(T - 1)

    # ---------------- selection scalars ----------------
    def psum_scalar(src_col, dst):  # [P,1] -> [1,1]
        pt = psum.tile([1, 1], F32, name="pss", tag="pss")
        nc.tensor.matmul(pt[:, :], src_col, ones_col[:, :], start=True, stop=True)
        ts(dst, pt[:, :], 1.0, OP.mult)

    def bcast_col(src_sc):  # [1,1] -> [P,1]
        bc = psum.tile([P, 1], F32, name="bcc", tag="bcc")
        nc.tensor.matmul(bc[:, :], ones_row[:, :], src_sc, start=True, stop=True)
        bcs = nt([P, 1])
        s.activation(bcs[:, :], bc[:, :], AF.Copy)
        return bcs

    mi = MACC[:, :].bitcast(I32)
    POSM, NEGY = AW, AH  # reuse dead prep tiles
    ts(POSM[:, :], MACC[:, :], 3.0, OP.is_le, accum=acc_np[:, :])
    npos_t = SC[:, 0:1]
    psum_scalar(acc_np[:, :], npos_t)
    stt(NEGY[:, :], POSM[:, :], POS_OFF, MACC[:, :], OP.mult, OP.add)

    # k = min(4*npos, N-npos)
    k_t = SC[:, 1:2]
    kA, kB = nt([1, 1]), nt([1, 1])
    ts(kA[:, :], npos_t, 4.0, OP.mult)
    ts(kB[:, :], npos_t, -1.0, OP.mult, float(N), OP.add)
    tt(k_t, kA[:, :], kB[:, :], OP.min)

    # decode metadata from MACC
    tcf, XL = ACX, ACY  # reuse
    xqi = nt([P, FD], I32)
    ts(xqi[:, :], mi, 31, OP.bitwise_and)
    ts(tcf[:, :], xqi[:, :], 1.0, OP.mult)          # tcode as f32
    ts(xqi[:, :], mi, 5, OP.arith_shift_right, 0x7F, OP.bitwise_and)
    ts(XL[:, :], xqi[:, :], 0.125, OP.mult, -8.0625, OP.add)

    # ---------------- fl_neg (dense) + cls_pos ----------------
    LSE, CE0 = DW, DH  # reuse
    s.activation(LSE[:, :], S_[:, :], AF.Ln)
    tt(CE0[:, :], LSE[:, :], X0[:, :], OP.subtract, eng=g)
    P0, u_, FLN = DCY, bufs[0]["ovx"], DCX  # FLN must outlive sl1 scratch
    s.activation(P0[:, :], CE0[:, :], AF.Exp, scale=-1.0)
    ts(u_[:, :], P0[:, :], -1.0, OP.mult, 1.0, OP.add)
    tt(FLN[:, :], u_[:, :], u_[:, :], OP.mult, eng=g)
    stt(FLN[:, :], FLN[:, :], 0.1, u_[:, :], OP.mult, OP.mult)
    tt(FLN[:, :], FLN[:, :], CE0[:, :], OP.mult, eng=g)

    CEP, PP = bufs[1]["ovx"], bufs[1]["ovy"]
    tt(CEP[:, :], LSE[:, :], XL[:, :], OP.subtract, eng=g)
    s.activation(PP[:, :], CEP[:, :], AF.Exp, scale=-1.0)
    ts(u_[:, :], PP[:, :], -1.0, OP.mult, 1.0, OP.add)
    tt(PP[:, :], u_[:, :], u_[:, :], OP.mult, eng=g)
    stt(PP[:, :], PP[:, :], 0.25, CEP[:, :], OP.mult, OP.mult)
    YPs = bufs[0]["YP"]
    stt(YPs[:, :], PP[:, :], 1.0, POSM[:, :], OP.mult, OP.mult,
        accum=acc_cp[:, :])
    psum_scalar(acc_cp[:, :], SC[:, 6:7])  # cls_pos

    # ---------------- gt lookup + smooth-L1 + bisection (woven) ----------
    # SELACC/sl1 DVE ops fill the bisection's PE round-trip latency.
    AXYf = DX0[:, :]
    AWHf = DX1[:, :]
    AXY = AXYf.bitcast(I32)
    AWH = AWHf.bitcast(I32)
    v.memset(AXY, 0)
    v.memset(AWH, 0)
    tcfF = tcf[:, :]

    mid, d_s = nt([1, 1], I32), nt([1, 1], I32)
    sel_i, nsel_i = nt([1, 1], I32), nt([1, 1], I32)
    sel_f = nt([1, 1])
    tmp1 = nt([1, 1], I32)
    geM = SA  # reuse

    _bis = [0]
    bis_bc = [None]

    def bisect_step_a():  # mid + bcast launch (cheap tiny ops + PE)
        tt(d_s[:, :], hi[:, :], lo[:, :], OP.subtract)
        ts(d_s[:, :], d_s[:, :], 1, OP.arith_shift_right)
        tt(mid[:, :], lo[:, :], d_s[:, :], OP.add)
        bc = psum.tile([P, 1], F32, name="bcc", tag="bcc")
        nc.tensor.matmul(bc[:, :], ones_row[:, :], mid[:, :].bitcast(F32),
                         start=True, stop=True)
        return bc

    def bisect_step_b(bc):  # count + select + window update
        ts(geM[:, :], NEGY[:, :], bc[:, 0:1], OP.is_le, accum=acc_cnt[:, :])
        pt = psum.tile([1, 1], F32, name="psb", tag="psb")
        nc.tensor.matmul(pt[:, :], acc_cnt[:, :], ones_col[:, :],
                         start=True, stop=True)
        tt(sel_f[:, :], pt[:, :], k_t, OP.is_ge)
        ts(sel_i[:, :], sel_f[:, :], 1, OP.mult)
        ts(nsel_i[:, :], sel_i[:, :], -1, OP.mult, 1, OP.add)  # 1-sel
        tt(d_s[:, :], mid[:, :], hi[:, :], OP.subtract)
        tt(d_s[:, :], d_s[:, :], sel_i[:, :], OP.mult)
        tt(hi[:, :], hi[:, :], d_s[:, :], OP.add)
        tt(tmp1[:, :], mid[:, :], lo[:, :], OP.subtract)
        tt(tmp1[:, :], tmp1[:, :], nsel_i[:, :], OP.mult)
        tt(lo[:, :], lo[:, :], tmp1[:, :], OP.add)

    def weave():
        # one bisection half-step between chunks of SELACC/sl1 work
        if _bis[0] >= 2 * NBIS:
            return
        if _bis[0] % 2 == 0:
            bis_bc[0] = bisect_step_a()
        else:
            bisect_step_b(bis_bc[0])
        _bis[0] += 1

    for t in range(T):
        v._custom_dve(SELACC, out=AXYf, in0=tcfF, in1=AXYf,
                      s0=float(t), s1=QXY[:, t:t + 1].bitcast(F32))
        v._custom_dve(SELACC, out=AWHf, in0=tcfF, in1=AWHf,
                      s0=float(t), s1=QWH[:, t:t + 1].bitcast(F32))
        weave()

    # ---------------- unpack gt + smooth-L1 (pipelined over coords) -------
    Gx, Gy, Gw, Gh = (ANv[:, i, :] for i in range(4))  # reuse AN storage
    gq = xqi
    gq2 = nt([P, FD], I32)
    ts(gq[:, :], AXY, 16, OP.arith_shift_right)
    ts(Gx, gq[:, :], 0.03125, OP.mult)
    ts(gq2[:, :], AXY, 0xFFFF, OP.bitwise_and)
    ts(Gy, gq2[:, :], 0.03125, OP.mult)
    weave()
    ts(gq[:, :], AWH, 16, OP.arith_shift_right)
    ts(Gw, gq[:, :], 0.000244140625, OP.mult, -2.0, OP.add)
    ts(gq2[:, :], AWH, 0xFFFF, OP.bitwise_and)
    ts(Gh, gq2[:, :], 0.000244140625, OP.mult, -2.0, OP.add)
    weave()

    SL = CE0  # dead after FLN chain
    v.memset(SL[:, :], 0.0)
    # four independent residual/sl1 pipelines (per-coord tiles)
    Rs = [P0, bufs[1]["Y"], bufs[1]["RI"], bufs[1]["I"]]
    sas = [bufs[0]["I"], bufs[0]["RI"], bufs[0]["Y"], bufs[0]["YP"]]
    sqs = [bufs[0]["U"], bufs[1]["U"], bufs[1]["YP"], gq.bitcast(F32)]
    slts = [bufs[0]["ovx"], bufs[0]["ovy"], bufs[1]["ovx"], bufs[1]["ovy"]]
    tt(Rs[0][:, :], Gx, RBX[:, :], OP.mult, eng=g)
    tt(Rs[1][:, :], Gy, RBY[:, :], OP.mult, eng=g)
    tt(Rs[0][:, :], ALX[:, :], Rs[0][:, :], OP.subtract)
    tt(Rs[1][:, :], ALY[:, :], Rs[1][:, :], OP.subtract)
    tt(Rs[2][:, :], GWl[:, :], Gw, OP.subtract, eng=g)
    tt(Rs[3][:, :], GHl[:, :], Gh, OP.subtract, eng=g)
    weave()
    for kk in range(4):  # abs + square stage
        ts(sas[kk][:, :].bitcast(I32), Rs[kk][:, :].bitcast(I32),
           0x7FFFFFFF, OP.bitwise_and)
        tt(sqs[kk][:, :], Rs[kk][:, :], Rs[kk][:, :], OP.mult, eng=g)
        weave()
    for kk in range(4):  # piecewise combine: lin + slt*(0.5*sq - lin)
        lin = Rs[kk]
        ts(lin[:, :], sas[kk][:, :], -0.5, OP.add)
        ts(slts[kk][:, :], sas[kk][:, :], 1.0, OP.is_lt)
        stt(sqs[kk][:, :], sqs[kk][:, :], 0.5, lin[:, :], OP.mult, OP.subtract)
        tt(sqs[kk][:, :], sqs[kk][:, :], slts[kk][:, :], OP.mult, eng=g)
        tt(sqs[kk][:, :], sqs[kk][:, :], lin[:, :], OP.add, eng=g)
        weave()
    tt(SL[:, :], sqs[0][:, :], sqs[1][:, :], OP.add, eng=g)
    tt(sqs[2][:, :], sqs[2][:, :], sqs[3][:, :], OP.add, eng=g)
    weave()
    tt(SL[:, :], SL[:, :], sqs[2][:, :], OP.add, eng=g)
    while _bis[0] < 2 * NBIS:
        weave()

    geM2 = bufs[1]["ovy"]
    YPs2 = bufs[1]["ovx"]
    acc_cnt2, acc_sum2 = nt([P, 1]), nt([P, 1])
    bc_lo = bcast_col(lo[:, :].bitcast(F32))
    bc_hi = bcast_col(hi[:, :].bitcast(F32))
    ts(geM[:, :], NEGY[:, :], bc_lo[:, 0:1], OP.is_le, accum=acc_cnt[:, :])
    ts(geM2[:, :], NEGY[:, :], bc_hi[:, 0:1], OP.is_le, accum=acc_cnt2[:, :])
    psum_scalar(acc_cnt[:, :], SC[:, 2:3])
    psum_scalar(acc_cnt2[:, :], SC[:, 4:5])
    stt(YPs[:, :], FLN[:, :], 1.0, geM[:, :], OP.mult, OP.mult,
        accum=acc_sum[:, :])
    stt(YPs2[:, :], FLN[:, :], 1.0, geM2[:, :], OP.mult, OP.mult,
        accum=acc_sum2[:, :])
    psum_scalar(acc_sum[:, :], SC[:, 3:4])
    psum_scalar(acc_sum2[:, :], SC[:, 5:6])

    stt(bufs[1]["Y"][:, :], SL[:, :], 1.0, POSM[:, :], OP.mult, OP.mult,
        accum=acc_sl[:, :])
    psum_scalar(acc_sl[:, :], SC[:, 7:8])  # sl1 sum
    nc.sync.dma_start(out_d[:, :], SC[:, :])
    ctx.close()


def _make_in_maps(cls_output, reg_output, anchors, target_boxes, target_labels):
    B = cls_output.shape[0]
    import ml_dtypes
    bf16 = ml_dtypes.bfloat16
    in_maps = []
    tcode = np.arange(T, dtype=np.int32)[:, None, None]
    # anchor-minor layout: a = f*128 + p  ->  [p, f]
    anc_pf = np.ascontiguousarray(
        np.asarray(anchors, np.float32).reshape(FD, P, 4).swapaxes(0, 1)
        .reshape(P, 4 * FD))
    for b in range(B):
        cls_b = np.asarray(cls_output[b], dtype=np.float32)
        labels_b = np.asarray(target_labels[b]).astype(np.int64)
        # 7-bit quantized gathered logits + 5-bit target index, [T, P, FD]
        xl = cls_b[:, labels_b].T.reshape(T, FD, P).swapaxes(1, 2)
        xq = np.clip(np.floor(xl * 8.0 + 64.5), 0, 127).astype(np.int32)
        meta = (xq << 5) | tcode
        in_maps.append({
            "clsT": np.ascontiguousarray(cls_b.T).astype(bf16),
            "meta": np.ascontiguousarray(meta),
            "reg": np.ascontiguousarray(
                np.asarray(reg_output[b], np.float32).reshape(FD, P, 4)
                .swapaxes(0, 1).reshape(P, 4 * FD)),
            "anc": anc_pf,
            "x0": np.ascontiguousarray(cls_b[:, 0].reshape(FD, P).T),
            "tb": np.ascontiguousarray(target_boxes[b], dtype=np.float32),
        })
    return in_maps


def kernel(cls_output, reg_output, anchors, target_boxes, target_labels):
    global _compiled
    if _compiled is None:
        _compiled = _build()
    nc = _compiled
    B = cls_output.shape[0]
    in_maps = _make_in_maps(cls_output, reg_output, anchors, target_boxes,
                            target_labels)
    res = bass_utils.run_bass_kernel_spmd(nc, in_maps, core_ids=list(range(B)))

    cls_l = np.zeros(B, np.float32)
    reg_l = np.zeros(B, np.float32)
    npos_a = np.zeros(B, np.int64)
    for b in range(B):
        sc = res.results[b]["out"][0]
        npos, k = float(sc[0]), float(sc[1])
        c_lo, s_lo, c_hi, s_hi = float(sc[2]), float(sc[3]), float(sc[4]), float(sc[5])
        cls_pos, sl1s = float(sc[6]), float(sc[7])
        # fractional blend on the bisection plateau: exactly k negatives
        if c_hi > c_lo:
            frac = (k - c_lo) / (c_hi - c_lo)
        else:
            frac = 0.0
        cls_neg = s_lo + frac * (s_hi - s_lo)
        total = max(npos + k, 1.0)
        cls_l[b] = np.float32((cls_pos + cls_neg) / total)
        reg_l[b] = np.float32(sl1s / (npos + 1e-6))
        npos_a[b] = int(round(npos))

    total_pos = np.int32(npos_a.sum())
    cls_final = np.float32(cls_l.mean())
    reg_final = np.float32(reg_l.mean()) if total_pos > 0 else np.float32(0.0)
    reg_weight = np.float32(min(1.0, float(total_pos) / (100.0 * B)))
    total_loss = np.float32(cls_final + reg_weight * 1.0 * reg_final)
    return (total_loss, cls_final, reg_final, np.int32(total_pos))


# revision 22
# speedup vs baseline: 1.3132x; 1.0437x over previous
"""Trainium2 Bass kernel for nn_DetectionLoss (anchor matching + focal/smooth-L1).

Pure data parallelism: image b runs on core b (B=8). Each core emits 8 partial
scalars; the host combines them (same final reduction as the reference).

Device algorithm (N=65536 anchors, T=32 targets, C=80 classes), v2:
  - Work with the inverted score y = U/I (U = Sa+Sb+eps, I = intersection);
    y is strictly decreasing in IoU, so argmax-IoU = argmin-y and
    iou >= 0.5  <=>  y <= 3 (exactly the reference's threshold algebra).
  - Per t: y computed in ONE scalar_tensor_tensor op ((SA+sbe_t)/I); overlap
    widths via one fused custom DVE op each (relu(min-max)+1e-15, the epsilon
    keeps y finite so no NaN/inf enters the min-reduce); the packed min over t
    carries 12 bits of metadata (7-bit quantized matched-class logit + 5-bit
    target index) in the fp32 mantissa low bits; I-mult and min-accumulate run
    on the otherwise-idle GPSIMD engine.
  - Classification: cls arrives host-transposed as bf16 [C, N]; exp on the
    scalar engine, per-anchor softmax denominator S via PE matmul with a ones
    vector (contraction over the 80 class partitions), landed into anchor-major
    layout by tiny PSUM->SBUF DMAs. ce = log S - x; focal sums via
    tensor_scalar/stt accum_out fused row-sums + PE column-sum.
  - Hard negatives: rank by y ascending among non-positives; k-th threshold by
    bisection on the int32 bit pattern of y (positive floats are bit-ordered),
    host blends the boundary plateau fractionally (exact top-k to ~1e-4).
  - Matched-gt lookup for smooth-L1: 16+16-bit packed gt quantities selected
    per anchor by a custom select-or-accumulate op over the 5-bit target code
    (disjoint one-hot masks => bitwise-OR accumulate).

Host-side prep (sharding/layout only, no loss math): per-image transpose+cast
of cls to bf16 [C, N], gather of the 32 labeled logit columns quantized into
the 12-bit metadata words, and the final 8->1 scalar reduction/blend.
"""

import sys, os

for _p in ("/opt/trn_rl_repo",):
    if _p not in sys.path:
        sys.path.insert(0, _p)

import numpy as np

import concourse.bass as bass
import concourse.bacc as bacc
import concourse.mybir as mybir
from concourse.tile import TileContext
from concourse import bass_utils

F32 = mybir.dt.float32
BF16 = mybir.dt.bfloat16
I32 = mybir.dt.int32
OP = mybir.AluOpType
AF = mybir.ActivationFunctionType

N, C, T = 65536, 80, 32
P, FD = 128, 512  # anchor a = p*FD + f
NCORES = 8
NBIS = 14
CCH = 4096        # anchors per cls-stream chunk
NCH = N // CCH
Y_LO_BITS = 0x40400000   # bits(3.0)
Y_HI_BITS = 0x5D000000   # ~5.8e17, above any finite y (eps=1e-6 keeps y <~ 1.5e17)
POS_OFF = 1e35           # added to y of positives so they rank last

_compiled = None


def _register_dve_op(name, spec):
    from concourse import dve_ops as DOPS
    from concourse.dve_spec import lower
    from concourse.dve_table_gen import DveOpSpec
    if name in DOPS._SUB_OPCODE_FOR_NAME:
        return next(o for o in DOPS.OPS if o.name == name)
    DOPS.OPS.append(DOPS.DveOp(name, spec, False, {}))
    DOPS._SUB_OPCODE_FOR_NAME[name] = DOPS._CUSTOM_DVE_ROW_BASE + len(DOPS.OPS) - 1
    DOPS.CUSTOM_DVE_SPECS[name] = spec
    opc = DOPS.get_dve_sub_opcode(name)
    shas = {}
    for ver in ("v3", "v4"):
        shas[ver] = DveOpSpec(name=name, opcode=opc, uops=lower(spec, ver=ver),
                              rd1_en=DOPS.has_src1(spec)).sha(ver)
    DOPS.OPS[-1] = DOPS.DveOp(name, spec, False, shas)
    return DOPS.OPS[-1]


def _get_ops():
    import numpy as _np
    from concourse.dve_spec import (Spec, Src0, Src1, C0, C1, C2, Zero, relu,
                                    minn, maxx, select, eq, Bin, AluOp)

    def _bits(a):
        return _np.asarray(a, _np.float32).view(_np.int32)

    def _ovlpe_ref(in0, in1, s0, s1, imm2):
        return _np.maximum(_np.minimum(in0, s0) - _np.maximum(in1, s1), 0.0) + imm2

    def _pack_ref(in0, in1, s0, s1):
        m = _bits(s0)
        return ((_bits(in0) & ~m) | _bits(in1)).view(_np.float32)

    def _selacc_ref(in0, in1, s0, s1):
        pick = _np.where(_np.asarray(in0, _np.float32) == _np.float32(s0),
                         _np.broadcast_to(_bits(s1), in0.shape), 0)
        return (_bits(in1) | pick).view(_np.float32)

    # overlap width + tiny epsilon: relu(min(Src0,C0) - max(Src1,C1)) + C2
    ovlpe = _register_dve_op(
        "ANT_DL_OVLPE",
        Spec(body=Bin(AluOp.ADD, relu(minn(Src0, C0) - maxx(Src1, C1)), C2),
             reference=lambda in0, in1, s0, s1, imm2: _ovlpe_ref(in0, in1, s0, s1, imm2)))
    # clear low-12 bits of Src0, OR in Src1 (metadata)
    pack = _register_dve_op(
        "ANT_DL_PACK",
        Spec(body=Bin(AluOp.BITWISE_OR,
                      Bin(AluOp.BITWISE_XOR, Src0,
                          Bin(AluOp.BITWISE_AND, Src0, C0)), Src1),
             reference=lambda in0, in1, s0, s1, imm2: _pack_ref(in0, in1, s0, s1)))
    # select-or-accumulate: Src1 | (Src0 == C0 ? C1 : 0)   (disjoint masks)
    selacc = _register_dve_op(
        "ANT_DL_SELACC",
        Spec(body=Bin(AluOp.BITWISE_OR, Src1, select(eq(Src0, C0), C1, Zero)),
             reference=lambda in0, in1, s0, s1, imm2: _selacc_ref(in0, in1, s0, s1)))
    return ovlpe, pack, selacc


def _build():
    nc = bacc.Bacc("TRN2", target_bir_lowering=False, debug=False,
                   enable_asserts=False, num_devices=NCORES)
    clsT_d = nc.dram_tensor("clsT", [C, N], BF16, kind="ExternalInput")
    meta_d = nc.dram_tensor("meta", [T, P, FD], I32, kind="ExternalInput")
    reg_d = nc.dram_tensor("reg", [P, 4 * FD], F32, kind="ExternalInput")
    anc_d = nc.dram_tensor("anc", [P, 4 * FD], F32, kind="ExternalInput")
    x0_d = nc.dram_tensor("x0", [P, FD], F32, kind="ExternalInput")
    tb_d = nc.dram_tensor("tb", [T, 4], F32, kind="ExternalInput")
    out_d = nc.dram_tensor("out", [1, 16], F32, kind="ExternalOutput")

    with TileContext(nc) as tc:
        _emit(nc, tc, clsT_d, meta_d, reg_d, anc_d, x0_d, tb_d, out_d)
    nc.compile()
    return nc


def _emit(nc, tc, clsT_d, meta_d, reg_d, anc_d, x0_d, tb_d, out_d):
    import contextlib
    ctx = contextlib.ExitStack()
    pool = ctx.enter_context(tc.tile_pool(name="main", bufs=1))
    cpool = ctx.enter_context(tc.tile_pool(name="cls", bufs=2))
    mpool = ctx.enter_context(tc.tile_pool(name="meta", bufs=2))
    psum = ctx.enter_context(tc.tile_pool(name="ps", bufs=1, space="PSUM"))
    pssum = ctx.enter_context(tc.tile_pool(name="psS", bufs=2, space="PSUM"))
    v, s, g = nc.vector, nc.scalar, nc.gpsimd

    def ts(out, in0, s1, op0, s2=None, op1=None, accum=None, eng=v):
        if accum is not None and op1 is None:
            op1 = OP.add  # accum reduce op rides in op1
        kw = dict(scalar2=s2) if op1 is None else dict(scalar2=s2, op1=op1)
        if accum is not None:
            kw["accum_out"] = accum
        return eng.tensor_scalar(out=out, in0=in0, scalar1=s1, op0=op0, **kw)

    def tt(out, in0, in1, op, eng=v):
        return eng.tensor_tensor(out=out, in0=in0, in1=in1, op=op)

    def stt(out, in0, sc, in1, op0, op1, accum=None, eng=v):
        kw = {} if accum is None else {"accum_out": accum}
        return eng.scalar_tensor_tensor(out=out, in0=in0, scalar=sc, in1=in1,
                                        op0=op0, op1=op1, **kw)

    _ctr = [0]

    def nt(shape, dt=F32):
        _ctr[0] += 1
        return pool.tile(shape, dt, name=f"tl{_ctr[0]}", tag=f"tl{_ctr[0]}")

    OVLPE, PACKOP, SELACC = _get_ops()

    # ---------------- input DMAs (priority order: t-loop deps first) -------
    TB = nt([P, 4 * T])
    nc.sync.dma_start(TB[:, :], tb_d.rearrange("t c -> (t c)")[None, :]
                      .broadcast_to([P, 4 * T]))
    RG, AN = nt([P, 4 * FD]), nt([P, 4 * FD])
    nc.sync.dma_start(AN[:, :], anc_d[:, :])
    nc.sync.dma_start(RG[:, :], reg_d[:, :])
    X0 = nt([P, FD])
    nc.sync.dma_start(X0[:, :], x0_d[:, :])

    # ---------------- target prep (tiny) ----------------
    TBv = TB[:, :].rearrange("p (t c) -> p t c", c=4)
    tx0, ty0 = TBv[:, :, 0], TBv[:, :, 1]
    tx1, ty1 = TBv[:, :, 2], TBv[:, :, 3]
    WB, HB, SBE = nt([P, T]), nt([P, T]), nt([P, T])
    GCX, GCY, LNW, LNH = nt([P, T]), nt([P, T]), nt([P, T]), nt([P, T])
    tt(WB[:, :], tx1, tx0, OP.subtract)
    tt(HB[:, :], ty1, ty0, OP.subtract)
    tmpT = nt([P, T])
    tt(tmpT[:, :], WB[:, :], HB[:, :], OP.mult)
    ts(SBE[:, :], tmpT[:, :], 1e-6, OP.add)
    stt(GCX[:, :], WB[:, :], 0.5, tx0, OP.mult, OP.add)
    stt(GCY[:, :], HB[:, :], 0.5, ty0, OP.mult, OP.add)
    s.activation(LNW[:, :], WB[:, :], AF.Ln)
    s.activation(LNH[:, :], HB[:, :], AF.Ln)
    # 16+16-bit packed gt coords (x<<16|y), (lnw<<16|lnh)
    QXY, QWH = nt([P, T], I32), nt([P, T], I32)
    q0, q1 = nt([P, T], I32), nt([P, T], I32)
    ts(q0[:, :], GCX[:, :], 32.0, OP.mult)
    ts(q1[:, :], GCY[:, :], 32.0, OP.mult)
    qt = nt([P, T], I32)
    ts(qt[:, :], q0[:, :], 65536, OP.mult)
    tt(QXY[:, :], qt[:, :], q1[:, :], OP.add)
    ts(q0[:, :], LNW[:, :], 4096.0, OP.mult, 8192.0, OP.add)
    ts(q1[:, :], LNH[:, :], 4096.0, OP.mult, 8192.0, OP.add)
    ts(qt[:, :], q0[:, :], 65536, OP.mult)
    tt(QWH[:, :], qt[:, :], q1[:, :], OP.add)

    # constants / accumulators early (fills the input-DMA wait)
    SC = nt([1, 16])
    v.memset(SC[:, :], 0.0)
    ones_col = nt([P, 1])
    v.memset(ones_col[:, :], 1.0)
    ones_row = nt([1, P])
    v.memset(ones_row[:, :], 1.0)
    acc_np, acc_cnt, acc_sum, acc_cp, acc_sl = (nt([P, 1]) for _ in range(5))
    MACC = nt([P, FD])
    v.memset(MACC[:, :].bitcast(I32), Y_HI_BITS + 0x3FFFFF)
    MSKC = nt([P, 1], I32)
    v.memset(MSKC[:, :], 0xFFF)
    lo, hi = nt([1, 1], I32), nt([1, 1], I32)
    v.memset(lo[:, :], Y_LO_BITS)
    v.memset(hi[:, :], Y_HI_BITS)

    # ---------------- anchor prep ----------------
    RGv = RG[:, :].rearrange("p (f c) -> p c f", c=4)
    ANv = AN[:, :].rearrange("p (f c) -> p c f", c=4)
    rg0, rg1, rg2, rg3 = (RGv[:, i, :] for i in range(4))
    ax0, ay0, ax1, ay1 = (ANv[:, i, :] for i in range(4))

    AW, AH, ACX, ACY = nt([P, FD]), nt([P, FD]), nt([P, FD]), nt([P, FD])
    tt(AW[:, :], ax1, ax0, OP.subtract)
    tt(AH[:, :], ay1, ay0, OP.subtract)
    stt(ACX[:, :], AW[:, :], 0.5, ax0, OP.mult, OP.add)
    stt(ACY[:, :], AH[:, :], 0.5, ay0, OP.mult, OP.add)

    # anc-only reg-target helpers first (reg DMA still in flight)
    AWE, AHE = nt([P, FD]), nt([P, FD])
    ts(AWE[:, :], AW[:, :], 1e-6, OP.add)
    ts(AHE[:, :], AH[:, :], 1e-6, OP.add)
    RBX, RBY, rsc = nt([P, FD]), nt([P, FD]), nt([P, FD])
    v.reciprocal_approx_accurate(out=RBX[:, :], in_=AWE[:, :], scratch=rsc[:, :])
    v.reciprocal_approx_accurate(out=RBY[:, :], in_=AHE[:, :], scratch=rsc[:, :])

    DW, DH = nt([P, FD]), nt([P, FD])
    s.activation(DW[:, :], rg2, AF.Exp)
    s.activation(DH[:, :], rg3, AF.Exp)
    tt(DW[:, :], DW[:, :], AW[:, :], OP.mult, eng=g)
    tt(DH[:, :], DH[:, :], AH[:, :], OP.mult, eng=g)
    DCX, DCY = nt([P, FD]), nt([P, FD])
    tt(DCX[:, :], AW[:, :], rg0, OP.mult, eng=g)
    tt(DCX[:, :], DCX[:, :], ACX[:, :], OP.add)
    tt(DCY[:, :], AH[:, :], rg1, OP.mult, eng=g)
    tt(DCY[:, :], DCY[:, :], ACY[:, :], OP.add)
    DX0, DY0, DX1, DY1, SA = (nt([P, FD]) for _ in range(5))
    stt(DX0[:, :], DW[:, :], -0.5, DCX[:, :], OP.mult, OP.add)
    stt(DX1[:, :], DW[:, :], 0.5, DCX[:, :], OP.mult, OP.add)
    stt(DY0[:, :], DH[:, :], -0.5, DCY[:, :], OP.mult, OP.add)
    stt(DY1[:, :], DH[:, :], 0.5, DCY[:, :], OP.mult, OP.add)
    tt(SA[:, :], DW[:, :], DH[:, :], OP.mult)
    ALX, ALY, GWl, GHl = nt([P, FD]), nt([P, FD]), nt([P, FD]), nt([P, FD])
    s.activation(GWl[:, :], AWE[:, :], AF.Ln)
    s.activation(GHl[:, :], AHE[:, :], AF.Ln)
    prep_late = [
        lambda: tt(ALX[:, :], ACX[:, :], RBX[:, :], OP.mult, eng=g),
        lambda: tt(ALX[:, :], ALX[:, :], rg0, OP.add, eng=g),
        lambda: tt(ALY[:, :], ACY[:, :], RBY[:, :], OP.mult, eng=g),
        lambda: tt(ALY[:, :], ALY[:, :], rg1, OP.add, eng=g),
        lambda: tt(GWl[:, :], GWl[:, :], rg2, OP.add, eng=g),
        lambda: tt(GHl[:, :], GHl[:, :], rg3, OP.add, eng=g),
    ]

    # ---------------- cls stream pieces (embedded in t-loop) --------------
    S_ = nt([P, FD])
    ones80 = nt([C, 1], BF16)
    v.memset(ones80[:, :], 1.0)
    MCH = CCH // P  # S columns per chunk

    def emit_cls_chunk(ch):
        CT = cpool.tile([C, CCH], BF16, name="ct", tag="ct")
        nc.sync.dma_start(CT[:, :], clsT_d[:, ch * CCH:(ch + 1) * CCH])
        s.activation(CT[:, :], CT[:, :], AF.Exp)
        pS = pssum.tile([P, MCH], F32, name="pS", tag="pS")
        for j in range(MCH):
            nc.tensor.matmul(pS[:, j:j + 1], CT[:, j * P:(j + 1) * P],
                             ones80[:, :], start=True, stop=True)
        s.activation(S_[:, ch * MCH:(ch + 1) * MCH], pS[:, :], AF.Copy)

    # ---------------- t-loop: packed min over targets (sw-pipelined) ------
    TPC = 4  # meta rows per streamed chunk
    NB = 2
    bufs = [dict(ovx=nt([P, FD]), ovy=nt([P, FD]), I=nt([P, FD]),
                 RI=nt([P, FD]), Y=nt([P, FD]), YP=nt([P, FD]),
                 U=nt([P, FD])) for _ in range(NB)]
    mtv = [None, None]

    def fetch_meta(c, eng=None):
        if c >= T // TPC:
            return
        MT = mpool.tile([P, TPC * FD], I32, name="mt", tag="mt")
        (eng or s).dma_start(MT[:, :].rearrange("p (t f) -> p t f", f=FD),
                             meta_d[c * TPC:(c + 1) * TPC, :, :]
                             .rearrange("t p f -> p t f"))
        mtv[c % 2] = MT[:, :].rearrange("p (t f) -> p t f", f=FD)

    def emit_front(t):
        b = bufs[t % NB]
        if t % TPC == 1:
            fetch_meta((t - 1) // TPC + 1)  # prefetch next chunk
        v._custom_dve(OVLPE, out=b["ovx"][:, :], in0=DX1[:, :], in1=DX0[:, :],
                      s0=TB[:, 4 * t + 2:4 * t + 3],
                      s1=TB[:, 4 * t + 0:4 * t + 1], imm2=1e-6)
        v._custom_dve(OVLPE, out=b["ovy"][:, :], in0=DY1[:, :], in1=DY0[:, :],
                      s0=TB[:, 4 * t + 3:4 * t + 4],
                      s1=TB[:, 4 * t + 1:4 * t + 2], imm2=1e-6)
        s.activation(b["U"][:, :], SA[:, :], AF.Identity, bias=SBE[:, t:t + 1])
        tt(b["I"][:, :], b["ovx"][:, :], b["ovy"][:, :], OP.mult, eng=g)

    def emit_back(t):
        b = bufs[t % NB]
        v.reciprocal_approx_fast(out=b["RI"][:, :], in_=b["I"][:, :])
        tt(b["Y"][:, :], b["U"][:, :], b["RI"][:, :], OP.mult, eng=g)
        v._custom_dve(PACKOP, out=b["YP"][:, :], in0=b["Y"][:, :],
                      in1=mtv[(t // TPC) % 2][:, t % TPC, :].bitcast(F32),
                      s0=MSKC[:, :].bitcast(F32))
        tt(MACC[:, :], MACC[:, :], b["YP"][:, :], OP.min)

    fetch_meta(0, eng=nc.sync)
    emit_front(0)
    for t in range(1, T):
        emit_front(t)
        if (t - 1) % 2 == 0:
            emit_cls_chunk((t - 1) // 2)
        if prep_late:
            prep_late.pop(0)()
        emit_back(t - 1)
    emit_cls_chunk(15)
    emit_back(T - 1)

    # ---------------- selection scalars ----------------
    def psum_scalar(src_col, dst):  # [P,1] -> [1,1]
        pt = psum.tile([1, 1], F32, name="pss", tag="pss")
        nc.tensor.matmul(pt[:, :], src_col, ones_col[:, :], start=True, stop=True)
        ts(dst, pt[:, :], 1.0, OP.mult)

    def bcast_col(src_sc):  # [1,1] -> [P,1]
        bc = psum.tile([P, 1], F32, name="bcc", tag="bcc")
        nc.tensor.matmul(bc[:, :], ones_row[:, :], src_sc, start=True, stop=True)
        bcs = nt([P, 1])
        s.activation(bcs[:, :], bc[:, :], AF.Copy)
        return bcs

    mi = MACC[:, :].bitcast(I32)
    POSM, NEGY = AW, AH  # reuse dead prep tiles
    ts(POSM[:, :], MACC[:, :], 3.0, OP.is_le, accum=acc_np[:, :])
    npos_t = SC[:, 0:1]
    psum_scalar(acc_np[:, :], npos_t)
    stt(NEGY[:, :], POSM[:, :], POS_OFF, MACC[:, :], OP.mult, OP.add)

    # k = min(4*npos, N-npos)
    k_t = SC[:, 1:2]
    kA, kB = nt([1, 1]), nt([1, 1])
    ts(kA[:, :], npos_t, 4.0, OP.mult)
    ts(kB[:, :], npos_t, -1.0, OP.mult, float(N), OP.add)
    tt(k_t, kA[:, :], kB[:, :], OP.min)

    # decode metadata from MACC
    tcf, XL = ACX, ACY  # reuse
    xqi = nt([P, FD], I32)
    ts(xqi[:, :], mi, 31, OP.bitwise_and)
    ts(tcf[:, :], xqi[:, :], 1.0, OP.mult)          # tcode as f32
    ts(xqi[:, :], mi, 5, OP.arith_shift_right, 0x7F, OP.bitwise_and)
    ts(XL[:, :], xqi[:, :], 0.125, OP.mult, -8.0625, OP.add)

    # ---------------- fl_neg (dense) + cls_pos ----------------
    LSE, CE0 = DW, DH  # reuse
    s.activation(LSE[:, :], S_[:, :], AF.Ln)
    tt(CE0[:, :], LSE[:, :], X0[:, :], OP.subtract, eng=g)
    P0, u_, FLN = DCY, bufs[0]["ovx"], DCX  # FLN must outlive sl1 scratch
    s.activation(P0[:, :], CE0[:, :], AF.Exp, scale=-1.0)
    ts(u_[:, :], P0[:, :], -1.0, OP.mult, 1.0, OP.add)
    tt(FLN[:, :], u_[:, :], u_[:, :], OP.mult, eng=g)
    stt(FLN[:, :], FLN[:, :], 0.1, u_[:, :], OP.mult, OP.mult)
    tt(FLN[:, :], FLN[:, :], CE0[:, :], OP.mult, eng=g)

    CEP, PP = bufs[1]["ovx"], bufs[1]["ovy"]
    tt(CEP[:, :], LSE[:, :], XL[:, :], OP.subtract, eng=g)
    s.activation(PP[:, :], CEP[:, :], AF.Exp, scale=-1.0)
    ts(u_[:, :], PP[:, :], -1.0, OP.mult, 1.0, OP.add)
    tt(PP[:, :], u_[:, :], u_[:, :], OP.mult, eng=g)
    stt(PP[:, :], PP[:, :], 0.25, CEP[:, :], OP.mult, OP.mult)
    YPs = bufs[0]["YP"]
    stt(YPs[:, :], PP[:, :], 1.0, POSM[:, :], OP.mult, OP.mult,
        accum=acc_cp[:, :])
    psum_scalar(acc_cp[:, :], SC[:, 6:7])  # cls_pos

    # ---------------- gt lookup + smooth-L1 + bisection (woven) ----------
    # SELACC/sl1 DVE ops fill the bisection's PE round-trip latency.
    AXYf = DX0[:, :]
    AWHf = DX1[:, :]
    AXY = AXYf.bitcast(I32)
    AWH = AWHf.bitcast(I32)
    v.memset(AXY, 0)
    v.memset(AWH, 0)
    tcfF = tcf[:, :]

    mid, d_s = nt([1, 1], I32), nt([1, 1], I32)
    sel_i, nsel_i = nt([1, 1], I32), nt([1, 1], I32)
    sel_f = nt([1, 1])
    tmp1 = nt([1, 1], I32)
    geM = SA  # reuse

    _bis = [0]
    bis_bc = [None]

    def bisect_step_a():  # mid + bcast launch (cheap tiny ops + PE)
        tt(d_s[:, :], hi[:, :], lo[:, :], OP.subtract)
        ts(d_s[:, :], d_s[:, :], 1, OP.arith_shift_right)
        tt(mid[:, :], lo[:, :], d_s[:, :], OP.add)
        bc = psum.tile([P, 1], F32, name="bcc", tag="bcc")
        nc.tensor.matmul(bc[:, :], ones_row[:, :], mid[:, :].bitcast(F32),
                         start=True, stop=True)
        return bc

    def bisect_step_b(bc):  # count + select + window update
        ts(geM[:, :], NEGY[:, :], bc[:, 0:1], OP.is_le, accum=acc_cnt[:, :])
        pt = psum.tile([1, 1], F32, name="psb", tag="psb")
        nc.tensor.matmul(pt[:, :], acc_cnt[:, :], ones_col[:, :],
                         start=True, stop=True)
        tt(sel_f[:, :], pt[:, :], k_t, OP.is_ge)
        ts(sel_i[:, :], sel_f[:, :], 1, OP.mult)
        ts(nsel_i[:, :], sel_i[:, :], -1, OP.mult, 1, OP.add)  # 1-sel
        tt(d_s[:, :], mid[:, :], hi[:, :], OP.subtract)
        tt(d_s[:, :], d_s[:, :], sel_i[:, :], OP.mult)
        tt(hi[:, :], hi[:, :], d_s[:, :], OP.add)
        tt(tmp1[:, :], mid[:, :], lo[:, :], OP.subtract)
        tt(tmp1[:, :], tmp1[:, :], nsel_i[:, :], OP.mult)
        tt(lo[:, :], lo[:, :], tmp1[:, :], OP.add)

    def weave():
        # one bisection half-step between chunks of SELACC/sl1 work
        if _bis[0] >= 2 * NBIS:
            return
        if _bis[0] % 2 == 0:
            bis_bc[0] = bisect_step_a()
        else:
            bisect_step_b(bis_bc[0])
        _bis[0] += 1

    for t in range(T):
        v._custom_dve(SELACC, out=AXYf, in0=tcfF, in1=AXYf,
                      s0=float(t), s1=QXY[:, t:t + 1].bitcast(F32))
        v._custom_dve(SELACC, out=AWHf, in0=tcfF, in1=AWHf,
                      s0=float(t), s1=QWH[:, t:t + 1].bitcast(F32))
        weave()

    # ---------------- unpack gt + smooth-L1 (pipelined over coords) -------
    Gx, Gy, Gw, Gh = (ANv[:, i, :] for i in range(4))  # reuse AN storage
    gq = xqi
    gq2 = nt([P, FD], I32)
    ts(gq[:, :], AXY, 16, OP.arith_shift_right)
    ts(Gx, gq[:, :], 0.03125, OP.mult)
    ts(gq2[:, :], AXY, 0xFFFF, OP.bitwise_and)
    ts(Gy, gq2[:, :], 0.03125, OP.mult)
    weave()
    GwR, GhR = nt([P, FD]), nt([P, FD])
    ts(gq[:, :], AWH, 16, OP.arith_shift_right)
    ts(GwR[:, :], gq[:, :], 0.000244140625, OP.mult)
    ts(gq2[:, :], AWH, 0xFFFF, OP.bitwise_and)
    ts(GhR[:, :], gq2[:, :], 0.000244140625, OP.mult)
    weave()

    SL = CE0  # dead after FLN chain
    v.memset(SL[:, :], 0.0)
    # four independent residual/sl1 pipelines (per-coord tiles)
    Rs = [P0, bufs[1]["Y"], bufs[1]["RI"], bufs[1]["I"]]
    sas = [bufs[0]["I"], bufs[0]["RI"], bufs[0]["Y"], bufs[0]["YP"]]
    sqs = [bufs[0]["U"], bufs[1]["U"], bufs[1]["YP"], gq.bitcast(F32)]
    slts = [bufs[0]["ovx"], bufs[0]["ovy"], bufs[1]["ovx"], bufs[1]["ovy"]]
    tt(Rs[0][:, :], Gx, RBX[:, :], OP.mult, eng=g)
    tt(Rs[1][:, :], Gy, RBY[:, :], OP.mult, eng=g)
    tt(Rs[0][:, :], ALX[:, :], Rs[0][:, :], OP.subtract)
    tt(Rs[1][:, :], ALY[:, :], Rs[1][:, :], OP.subtract)
    stt(Rs[2][:, :], GWl[:, :], 2.0, GwR[:, :], OP.add, OP.subtract)
    stt(Rs[3][:, :], GHl[:, :], 2.0, GhR[:, :], OP.add, OP.subtract)
    weave()
    for kk in range(4):  # abs + square stage
        ts(sas[kk][:, :].bitcast(I32), Rs[kk][:, :].bitcast(I32),
           0x7FFFFFFF, OP.bitwise_and)
        tt(sqs[kk][:, :], Rs[kk][:, :], Rs[kk][:, :], OP.mult, eng=g)
        weave()
    for kk in range(4):  # piecewise combine: lin + slt*(0.5*sq - lin)
        lin = Rs[kk]
        ts(lin[:, :], sas[kk][:, :], -0.5, OP.add)
        ts(slts[kk][:, :], sas[kk][:, :], 1.0, OP.is_lt)
        stt(sqs[kk][:, :], sqs[kk][:, :], 0.5, lin[:, :], OP.mult, OP.subtract)
        tt(sqs[kk][:, :], sqs[kk][:, :], slts[kk][:, :], OP.mult, eng=g)
        tt(sqs[kk][:, :], sqs[kk][:, :], lin[:, :], OP.add, eng=g)
        weave()
    tt(SL[:, :], sqs[0][:, :], sqs[1][:, :], OP.add, eng=g)
    tt(sqs[2][:, :], sqs[2][:, :], sqs[3][:, :], OP.add, eng=g)
    weave()
    tt(SL[:, :], SL[:, :], sqs[2][:, :], OP.add, eng=g)
    while _bis[0] < 2 * NBIS:
        weave()

    stt(bufs[1]["U"][:, :], SL[:, :], 1.0, POSM[:, :], OP.mult, OP.mult,
        accum=acc_sl[:, :])
    psum_scalar(acc_sl[:, :], SC[:, 7:8])  # sl1 sum
    geM2 = bufs[1]["ovy"]
    YPs2 = bufs[1]["ovx"]
    acc_cnt2, acc_sum2 = nt([P, 1]), nt([P, 1])
    bc_lo = bcast_col(lo[:, :].bitcast(F32))
    bc_hi = bcast_col(hi[:, :].bitcast(F32))
    ts(geM[:, :], NEGY[:, :], bc_lo[:, 0:1], OP.is_le, accum=acc_cnt[:, :])
    ts(geM2[:, :], NEGY[:, :], bc_hi[:, 0:1], OP.is_le, accum=acc_cnt2[:, :])
    psum_scalar(acc_cnt[:, :], SC[:, 2:3])
    psum_scalar(acc_cnt2[:, :], SC[:, 4:5])
    stt(YPs[:, :], FLN[:, :], 1.0, geM[:, :], OP.mult, OP.mult,
        accum=acc_sum[:, :])
    stt(YPs2[:, :], FLN[:, :], 1.0, geM2[:, :], OP.mult, OP.mult,
        accum=acc_sum2[:, :])
    psum_scalar(acc_sum[:, :], SC[:, 3:4])
    psum_scalar(acc_sum2[:, :], SC[:, 5:6])
    nc.sync.dma_start(out_d[:, :], SC[:, :])
    ctx.close()


def _make_in_maps(cls_output, reg_output, anchors, target_boxes, target_labels):
    B = cls_output.shape[0]
    import ml_dtypes
    bf16 = ml_dtypes.bfloat16
    in_maps = []
    tcode = np.arange(T, dtype=np.int32)[:, None, None]
    # anchor-minor layout: a = f*128 + p  ->  [p, f]
    anc_pf = np.ascontiguousarray(
        np.asarray(anchors, np.float32).reshape(FD, P, 4).swapaxes(0, 1)
        .reshape(P, 4 * FD))
    for b in range(B):
        cls_b = np.asarray(cls_output[b], dtype=np.float32)
        labels_b = np.asarray(target_labels[b]).astype(np.int64)
        # 7-bit quantized gathered logits + 5-bit target index, [T, P, FD]
        xl = cls_b[:, labels_b].T.reshape(T, FD, P).swapaxes(1, 2)
        xq = np.clip(np.floor(xl * 8.0 + 64.5), 0, 127).astype(np.int32)
        meta = (xq << 5) | tcode
        in_maps.append({
            "clsT": np.ascontiguousarray(cls_b.T).astype(bf16),
            "meta": np.ascontiguousarray(meta),
            "reg": np.ascontiguousarray(
                np.asarray(reg_output[b], np.float32).reshape(FD, P, 4)
                .swapaxes(0, 1).reshape(P, 4 * FD)),
            "anc": anc_pf,
            "x0": np.ascontiguousarray(cls_b[:, 0].reshape(FD, P).T),
            "tb": np.ascontiguousarray(target_boxes[b], dtype=np.float32),
        })
    return in_maps


def kernel(cls_output, reg_output, anchors, target_boxes, target_labels):
    global _compiled
    if _compiled is None:
        _compiled = _build()
    nc = _compiled
    B = cls_output.shape[0]
    in_maps = _make_in_maps(cls_output, reg_output, anchors, target_boxes,
                            target_labels)
    res = bass_utils.run_bass_kernel_spmd(nc, in_maps, core_ids=list(range(B)))

    cls_l = np.zeros(B, np.float32)
    reg_l = np.zeros(B, np.float32)
    npos_a = np.zeros(B, np.int64)
    for b in range(B):
        sc = res.results[b]["out"][0]
        npos, k = float(sc[0]), float(sc[1])
        c_lo, s_lo, c_hi, s_hi = float(sc[2]), float(sc[3]), float(sc[4]), float(sc[5])
        cls_pos, sl1s = float(sc[6]), float(sc[7])
        # fractional blend on the bisection plateau: exactly k negatives
        if c_hi > c_lo:
            frac = (k - c_lo) / (c_hi - c_lo)
        else:
            frac = 0.0
        cls_neg = s_lo + frac * (s_hi - s_lo)
        total = max(npos + k, 1.0)
        cls_l[b] = np.float32((cls_pos + cls_neg) / total)
        reg_l[b] = np.float32(sl1s / (npos + 1e-6))
        npos_a[b] = int(round(npos))

    total_pos = np.int32(npos_a.sum())
    cls_final = np.float32(cls_l.mean())
    reg_final = np.float32(reg_l.mean()) if total_pos > 0 else np.float32(0.0)
    reg_weight = np.float32(min(1.0, float(total_pos) / (100.0 * B)))
    total_loss = np.float32(cls_final + reg_weight * 1.0 * reg_final)
    return (total_loss, cls_final, reg_final, np.int32(total_pos))


# revision 23
# speedup vs baseline: 1.3328x; 1.0149x over previous
"""Trainium2 Bass kernel for nn_DetectionLoss (anchor matching + focal/smooth-L1).

Pure data parallelism: image b runs on core b (B=8). Each core emits 8 partial
scalars; the host combines them (same final reduction as the reference).

Device algorithm (N=65536 anchors, T=32 targets, C=80 classes), v2:
  - Work with the inverted score y = U/I (U = Sa+Sb+eps, I = intersection);
    y is strictly decreasing in IoU, so argmax-IoU = argmin-y and
    iou >= 0.5  <=>  y <= 3 (exactly the reference's threshold algebra).
  - Per t: y computed in ONE scalar_tensor_tensor op ((SA+sbe_t)/I); overlap
    widths via one fused custom DVE op each (relu(min-max)+1e-15, the epsilon
    keeps y finite so no NaN/inf enters the min-reduce); the packed min over t
    carries 12 bits of metadata (7-bit quantized matched-class logit + 5-bit
    target index) in the fp32 mantissa low bits; I-mult and min-accumulate run
    on the otherwise-idle GPSIMD engine.
  - Classification: cls arrives host-transposed as bf16 [C, N]; exp on the
    scalar engine, per-anchor softmax denominator S via PE matmul with a ones
    vector (contraction over the 80 class partitions), landed into anchor-major
    layout by tiny PSUM->SBUF DMAs. ce = log S - x; focal sums via
    tensor_scalar/stt accum_out fused row-sums + PE column-sum.
  - Hard negatives: rank by y ascending among non-positives; k-th threshold by
    bisection on the int32 bit pattern of y (positive floats are bit-ordered),
    host blends the boundary plateau fractionally (exact top-k to ~1e-4).
  - Matched-gt lookup for smooth-L1: 16+16-bit packed gt quantities selected
    per anchor by a custom select-or-accumulate op over the 5-bit target code
    (disjoint one-hot masks => bitwise-OR accumulate).

Host-side prep (sharding/layout only, no loss math): per-image transpose+cast
of cls to bf16 [C, N], gather of the 32 labeled logit columns quantized into
the 12-bit metadata words, and the final 8->1 scalar reduction/blend.
"""

import sys, os

for _p in ("/opt/trn_rl_repo",):
    if _p not in sys.path:
        sys.path.insert(0, _p)

import numpy as np

import concourse.bass as bass
import concourse.bacc as bacc
import concourse.mybir as mybir
from concourse.tile import TileContext
from concourse import bass_utils

F32 = mybir.dt.float32
BF16 = mybir.dt.bfloat16
I32 = mybir.dt.int32
OP = mybir.AluOpType
AF = mybir.ActivationFunctionType

N, C, T = 65536, 80, 32
P, FD = 128, 512  # anchor a = p*FD + f
NCORES = 8
NBIS = 12
CCH = 2048        # anchors per cls-stream chunk
NCH = N // CCH
Y_LO_BITS = 0x40400000   # bits(3.0)
Y_HI_BITS = 0x5D000000   # ~5.8e17, above any finite y (eps=1e-6 keeps y <~ 1.5e17)
POS_OFF = 1e35           # added to y of positives so they rank last

_compiled = None


def _register_dve_op(name, spec):
    from concourse import dve_ops as DOPS
    from concourse.dve_spec import lower
    from concourse.dve_table_gen import DveOpSpec
    if name in DOPS._SUB_OPCODE_FOR_NAME:
        return next(o for o in DOPS.OPS if o.name == name)
    DOPS.OPS.append(DOPS.DveOp(name, spec, False, {}))
    DOPS._SUB_OPCODE_FOR_NAME[name] = DOPS._CUSTOM_DVE_ROW_BASE + len(DOPS.OPS) - 1
    DOPS.CUSTOM_DVE_SPECS[name] = spec
    opc = DOPS.get_dve_sub_opcode(name)
    shas = {}
    for ver in ("v3", "v4"):
        shas[ver] = DveOpSpec(name=name, opcode=opc, uops=lower(spec, ver=ver),
                              rd1_en=DOPS.has_src1(spec)).sha(ver)
    DOPS.OPS[-1] = DOPS.DveOp(name, spec, False, shas)
    return DOPS.OPS[-1]


def _get_ops():
    import numpy as _np
    from concourse.dve_spec import (Spec, Src0, Src1, C0, C1, C2, Zero, relu,
                                    minn, maxx, select, eq, Bin, AluOp)

    def _bits(a):
        return _np.asarray(a, _np.float32).view(_np.int32)

    def _ovlpe_ref(in0, in1, s0, s1, imm2):
        return _np.maximum(_np.minimum(in0, s0) - _np.maximum(in1, s1), 0.0) + imm2

    def _pack_ref(in0, in1, s0, s1):
        m = _bits(s0)
        return ((_bits(in0) & ~m) | _bits(in1)).view(_np.float32)

    def _selacc_ref(in0, in1, s0, s1):
        pick = _np.where(_np.asarray(in0, _np.float32) == _np.float32(s0),
                         _np.broadcast_to(_bits(s1), in0.shape), 0)
        return (_bits(in1) | pick).view(_np.float32)

    # overlap width + tiny epsilon: relu(min(Src0,C0) - max(Src1,C1)) + C2
    ovlpe = _register_dve_op(
        "ANT_DL_OVLPE",
        Spec(body=Bin(AluOp.ADD, relu(minn(Src0, C0) - maxx(Src1, C1)), C2),
             reference=lambda in0, in1, s0, s1, imm2: _ovlpe_ref(in0, in1, s0, s1, imm2)))
    # clear low-12 bits of Src0, OR in Src1 (metadata)
    pack = _register_dve_op(
        "ANT_DL_PACK",
        Spec(body=Bin(AluOp.BITWISE_OR,
                      Bin(AluOp.BITWISE_XOR, Src0,
                          Bin(AluOp.BITWISE_AND, Src0, C0)), Src1),
             reference=lambda in0, in1, s0, s1, imm2: _pack_ref(in0, in1, s0, s1)))
    # select-or-accumulate: Src1 | (Src0 == C0 ? C1 : 0)   (disjoint masks)
    selacc = _register_dve_op(
        "ANT_DL_SELACC",
        Spec(body=Bin(AluOp.BITWISE_OR, Src1, select(eq(Src0, C0), C1, Zero)),
             reference=lambda in0, in1, s0, s1, imm2: _selacc_ref(in0, in1, s0, s1)))
    return ovlpe, pack, selacc


def _build():
    nc = bacc.Bacc("TRN2", target_bir_lowering=False, debug=False,
                   enable_asserts=False, num_devices=NCORES)
    clsT_d = nc.dram_tensor("clsT", [C, N], BF16, kind="ExternalInput")
    meta_d = nc.dram_tensor("meta", [T, P, FD], I32, kind="ExternalInput")
    reg_d = nc.dram_tensor("reg", [P, 4 * FD], F32, kind="ExternalInput")
    anc_d = nc.dram_tensor("anc", [P, 4 * FD], F32, kind="ExternalInput")
    x0_d = nc.dram_tensor("x0", [P, FD], F32, kind="ExternalInput")
    tb_d = nc.dram_tensor("tb", [T, 4], F32, kind="ExternalInput")
    out_d = nc.dram_tensor("out", [1, 16], F32, kind="ExternalOutput")

    with TileContext(nc) as tc:
        _emit(nc, tc, clsT_d, meta_d, reg_d, anc_d, x0_d, tb_d, out_d)
    nc.compile()
    return nc


def _emit(nc, tc, clsT_d, meta_d, reg_d, anc_d, x0_d, tb_d, out_d):
    import contextlib
    ctx = contextlib.ExitStack()
    pool = ctx.enter_context(tc.tile_pool(name="main", bufs=1))
    cpool = ctx.enter_context(tc.tile_pool(name="cls", bufs=2))
    mpool = ctx.enter_context(tc.tile_pool(name="meta", bufs=2))
    psum = ctx.enter_context(tc.tile_pool(name="ps", bufs=1, space="PSUM"))
    pssum = ctx.enter_context(tc.tile_pool(name="psS", bufs=2, space="PSUM"))
    v, s, g = nc.vector, nc.scalar, nc.gpsimd

    def ts(out, in0, s1, op0, s2=None, op1=None, accum=None, eng=v):
        if accum is not None and op1 is None:
            op1 = OP.add  # accum reduce op rides in op1
        kw = dict(scalar2=s2) if op1 is None else dict(scalar2=s2, op1=op1)
        if accum is not None:
            kw["accum_out"] = accum
        return eng.tensor_scalar(out=out, in0=in0, scalar1=s1, op0=op0, **kw)

    def tt(out, in0, in1, op, eng=v):
        return eng.tensor_tensor(out=out, in0=in0, in1=in1, op=op)

    def stt(out, in0, sc, in1, op0, op1, accum=None, eng=v):
        kw = {} if accum is None else {"accum_out": accum}
        return eng.scalar_tensor_tensor(out=out, in0=in0, scalar=sc, in1=in1,
                                        op0=op0, op1=op1, **kw)

    _ctr = [0]

    def nt(shape, dt=F32):
        _ctr[0] += 1
        return pool.tile(shape, dt, name=f"tl{_ctr[0]}", tag=f"tl{_ctr[0]}")

    OVLPE, PACKOP, SELACC = _get_ops()

    # ---------------- input DMAs (priority order: t-loop deps first) -------
    TB = nt([P, 4 * T])
    nc.sync.dma_start(TB[:, :], tb_d.rearrange("t c -> (t c)")[None, :]
                      .broadcast_to([P, 4 * T]))
    RG, AN = nt([P, 4 * FD]), nt([P, 4 * FD])
    nc.sync.dma_start(AN[:, :], anc_d[:, :])
    nc.sync.dma_start(RG[:, :], reg_d[:, :])
    X0 = nt([P, FD])
    nc.sync.dma_start(X0[:, :], x0_d[:, :])

    # ---------------- target prep (tiny) ----------------
    TBv = TB[:, :].rearrange("p (t c) -> p t c", c=4)
    tx0, ty0 = TBv[:, :, 0], TBv[:, :, 1]
    tx1, ty1 = TBv[:, :, 2], TBv[:, :, 3]
    WB, HB, SBE = nt([P, T]), nt([P, T]), nt([P, T])
    GCX, GCY, LNW, LNH = nt([P, T]), nt([P, T]), nt([P, T]), nt([P, T])
    tt(WB[:, :], tx1, tx0, OP.subtract)
    tt(HB[:, :], ty1, ty0, OP.subtract)
    tmpT = nt([P, T])
    tt(tmpT[:, :], WB[:, :], HB[:, :], OP.mult)
    ts(SBE[:, :], tmpT[:, :], 1e-6, OP.add)
    stt(GCX[:, :], WB[:, :], 0.5, tx0, OP.mult, OP.add)
    stt(GCY[:, :], HB[:, :], 0.5, ty0, OP.mult, OP.add)
    s.activation(LNW[:, :], WB[:, :], AF.Ln)
    s.activation(LNH[:, :], HB[:, :], AF.Ln)
    # 16+16-bit packed gt coords (x<<16|y), (lnw<<16|lnh)
    QXY, QWH = nt([P, T], I32), nt([P, T], I32)
    q0, q1 = nt([P, T], I32), nt([P, T], I32)
    ts(q0[:, :], GCX[:, :], 32.0, OP.mult)
    ts(q1[:, :], GCY[:, :], 32.0, OP.mult)
    qt = nt([P, T], I32)
    ts(qt[:, :], q0[:, :], 65536, OP.mult)
    tt(QXY[:, :], qt[:, :], q1[:, :], OP.add)
    ts(q0[:, :], LNW[:, :], 4096.0, OP.mult, 8192.0, OP.add)
    ts(q1[:, :], LNH[:, :], 4096.0, OP.mult, 8192.0, OP.add)
    ts(qt[:, :], q0[:, :], 65536, OP.mult)
    tt(QWH[:, :], qt[:, :], q1[:, :], OP.add)

    # constants / accumulators early (fills the input-DMA wait)
    SC = nt([1, 16])
    v.memset(SC[:, :], 0.0)
    ones_col = nt([P, 1])
    v.memset(ones_col[:, :], 1.0)
    ones_row = nt([1, P])
    v.memset(ones_row[:, :], 1.0)
    acc_np, acc_cnt, acc_sum, acc_cp, acc_sl = (nt([P, 1]) for _ in range(5))
    MACC = nt([P, FD])
    v.memset(MACC[:, :].bitcast(I32), Y_HI_BITS + 0x3FFFFF)
    MSKC = nt([P, 1], I32)
    v.memset(MSKC[:, :], 0xFFF)
    lo, hi = nt([1, 1], I32), nt([1, 1], I32)
    v.memset(lo[:, :], Y_LO_BITS)
    v.memset(hi[:, :], Y_HI_BITS)

    # ---------------- anchor prep ----------------
    RGv = RG[:, :].rearrange("p (f c) -> p c f", c=4)
    ANv = AN[:, :].rearrange("p (f c) -> p c f", c=4)
    rg0, rg1, rg2, rg3 = (RGv[:, i, :] for i in range(4))
    ax0, ay0, ax1, ay1 = (ANv[:, i, :] for i in range(4))

    AW, AH, ACX, ACY = nt([P, FD]), nt([P, FD]), nt([P, FD]), nt([P, FD])
    tt(AW[:, :], ax1, ax0, OP.subtract)
    tt(AH[:, :], ay1, ay0, OP.subtract)
    stt(ACX[:, :], AW[:, :], 0.5, ax0, OP.mult, OP.add)
    stt(ACY[:, :], AH[:, :], 0.5, ay0, OP.mult, OP.add)

    # anc-only reg-target helpers first (reg DMA still in flight)
    AWE, AHE = nt([P, FD]), nt([P, FD])
    ts(AWE[:, :], AW[:, :], 1e-6, OP.add)
    ts(AHE[:, :], AH[:, :], 1e-6, OP.add)
    RBX, RBY, rsc = nt([P, FD]), nt([P, FD]), nt([P, FD])
    v.reciprocal_approx_accurate(out=RBX[:, :], in_=AWE[:, :], scratch=rsc[:, :])
    v.reciprocal_approx_accurate(out=RBY[:, :], in_=AHE[:, :], scratch=rsc[:, :])

    DW, DH = nt([P, FD]), nt([P, FD])
    s.activation(DW[:, :], rg2, AF.Exp)
    s.activation(DH[:, :], rg3, AF.Exp)
    tt(DW[:, :], DW[:, :], AW[:, :], OP.mult, eng=g)
    tt(DH[:, :], DH[:, :], AH[:, :], OP.mult, eng=g)
    DCX, DCY = nt([P, FD]), nt([P, FD])
    tt(DCX[:, :], AW[:, :], rg0, OP.mult, eng=g)
    tt(DCX[:, :], DCX[:, :], ACX[:, :], OP.add)
    tt(DCY[:, :], AH[:, :], rg1, OP.mult, eng=g)
    tt(DCY[:, :], DCY[:, :], ACY[:, :], OP.add)
    DX0, DY0, DX1, DY1, SA = (nt([P, FD]) for _ in range(5))
    stt(DX0[:, :], DW[:, :], -0.5, DCX[:, :], OP.mult, OP.add)
    stt(DX1[:, :], DW[:, :], 0.5, DCX[:, :], OP.mult, OP.add)
    stt(DY0[:, :], DH[:, :], -0.5, DCY[:, :], OP.mult, OP.add)
    stt(DY1[:, :], DH[:, :], 0.5, DCY[:, :], OP.mult, OP.add)
    tt(SA[:, :], DW[:, :], DH[:, :], OP.mult)
    ALX, ALY, GWl, GHl = nt([P, FD]), nt([P, FD]), nt([P, FD]), nt([P, FD])
    s.activation(GWl[:, :], AWE[:, :], AF.Ln)
    s.activation(GHl[:, :], AHE[:, :], AF.Ln)
    prep_late = [
        lambda: tt(ALX[:, :], ACX[:, :], RBX[:, :], OP.mult, eng=g),
        lambda: tt(ALX[:, :], ALX[:, :], rg0, OP.add, eng=g),
        lambda: tt(ALY[:, :], ACY[:, :], RBY[:, :], OP.mult, eng=g),
        lambda: tt(ALY[:, :], ALY[:, :], rg1, OP.add, eng=g),
        lambda: tt(GWl[:, :], GWl[:, :], rg2, OP.add, eng=g),
        lambda: tt(GHl[:, :], GHl[:, :], rg3, OP.add, eng=g),
    ]

    # ---------------- cls stream pieces (embedded in t-loop) --------------
    S_ = nt([P, FD])
    ones80 = nt([C, 1], BF16)
    v.memset(ones80[:, :], 1.0)
    MCH = CCH // P  # S columns per chunk

    def emit_cls_chunk(ch):
        CT = cpool.tile([C, CCH], BF16, name="ct", tag="ct")
        nc.sync.dma_start(CT[:, :], clsT_d[:, ch * CCH:(ch + 1) * CCH])
        s.activation(CT[:, :], CT[:, :], AF.Exp)
        pS = pssum.tile([P, MCH], F32, name="pS", tag="pS")
        for j in range(MCH):
            nc.tensor.matmul(pS[:, j:j + 1], CT[:, j * P:(j + 1) * P],
                             ones80[:, :], start=True, stop=True)
        s.activation(S_[:, ch * MCH:(ch + 1) * MCH], pS[:, :], AF.Copy)

    # ---------------- t-loop: packed min over targets (sw-pipelined) ------
    TPC = 4  # meta rows per streamed chunk
    NB = 2
    bufs = [dict(ovx=nt([P, FD]), ovy=nt([P, FD]), I=nt([P, FD]),
                 RI=nt([P, FD]), Y=nt([P, FD]), YP=nt([P, FD]),
                 U=nt([P, FD])) for _ in range(NB)]
    mtv = [None, None]

    def fetch_meta(c, eng=None):
        if c >= T // TPC:
            return
        MT = mpool.tile([P, TPC * FD], I32, name="mt", tag="mt")
        (eng or s).dma_start(MT[:, :].rearrange("p (t f) -> p t f", f=FD),
                             meta_d[c * TPC:(c + 1) * TPC, :, :]
                             .rearrange("t p f -> p t f"))
        mtv[c % 2] = MT[:, :].rearrange("p (t f) -> p t f", f=FD)

    def emit_front(t):
        b = bufs[t % NB]
        if t % TPC == 1:
            fetch_meta((t - 1) // TPC + 1)  # prefetch next chunk
        v._custom_dve(OVLPE, out=b["ovx"][:, :], in0=DX1[:, :], in1=DX0[:, :],
                      s0=TB[:, 4 * t + 2:4 * t + 3],
                      s1=TB[:, 4 * t + 0:4 * t + 1], imm2=1e-6)
        v._custom_dve(OVLPE, out=b["ovy"][:, :], in0=DY1[:, :], in1=DY0[:, :],
                      s0=TB[:, 4 * t + 3:4 * t + 4],
                      s1=TB[:, 4 * t + 1:4 * t + 2], imm2=1e-6)
        s.activation(b["U"][:, :], SA[:, :], AF.Identity, bias=SBE[:, t:t + 1])
        tt(b["I"][:, :], b["ovx"][:, :], b["ovy"][:, :], OP.mult, eng=g)

    def emit_back(t):
        b = bufs[t % NB]
        v.reciprocal_approx_fast(out=b["RI"][:, :], in_=b["I"][:, :])
        tt(b["Y"][:, :], b["U"][:, :], b["RI"][:, :], OP.mult, eng=g)
        v._custom_dve(PACKOP, out=b["YP"][:, :], in0=b["Y"][:, :],
                      in1=mtv[(t // TPC) % 2][:, t % TPC, :].bitcast(F32),
                      s0=MSKC[:, :].bitcast(F32))
        tt(MACC[:, :], MACC[:, :], b["YP"][:, :], OP.min)

    fetch_meta(0, eng=nc.sync)
    emit_front(0)
    for t in range(1, T):
        emit_front(t)
        emit_cls_chunk(t - 1)
        if prep_late:
            prep_late.pop(0)()
        emit_back(t - 1)
    emit_cls_chunk(31)
    emit_back(T - 1)

    # ---------------- selection scalars ----------------
    def psum_scalar(src_col, dst):  # [P,1] -> [1,1]
        pt = psum.tile([1, 1], F32, name="pss", tag="pss")
        nc.tensor.matmul(pt[:, :], src_col, ones_col[:, :], start=True, stop=True)
        ts(dst, pt[:, :], 1.0, OP.mult)

    def bcast_col(src_sc):  # [1,1] -> [P,1]
        bc = psum.tile([P, 1], F32, name="bcc", tag="bcc")
        nc.tensor.matmul(bc[:, :], ones_row[:, :], src_sc, start=True, stop=True)
        bcs = nt([P, 1])
        s.activation(bcs[:, :], bc[:, :], AF.Copy)
        return bcs

    mi = MACC[:, :].bitcast(I32)
    POSM, NEGY = AW, AH  # reuse dead prep tiles
    ts(POSM[:, :], MACC[:, :], 3.0, OP.is_le, accum=acc_np[:, :])
    npos_t = SC[:, 0:1]
    psum_scalar(acc_np[:, :], npos_t)
    stt(NEGY[:, :], POSM[:, :], POS_OFF, MACC[:, :], OP.mult, OP.add)

    # k = min(4*npos, N-npos)
    k_t = SC[:, 1:2]
    kA, kB = nt([1, 1]), nt([1, 1])
    ts(kA[:, :], npos_t, 4.0, OP.mult)
    ts(kB[:, :], npos_t, -1.0, OP.mult, float(N), OP.add)
    tt(k_t, kA[:, :], kB[:, :], OP.min)

    # decode metadata from MACC
    tcf, XL = ACX, ACY  # reuse
    xqi = nt([P, FD], I32)
    ts(xqi[:, :], mi, 31, OP.bitwise_and)
    ts(tcf[:, :], xqi[:, :], 1.0, OP.mult)          # tcode as f32
    ts(xqi[:, :], mi, 5, OP.arith_shift_right, 0x7F, OP.bitwise_and)
    ts(XL[:, :], xqi[:, :], 0.125, OP.mult, -8.0625, OP.add)

    # ---------------- fl_neg (dense) + cls_pos ----------------
    LSE, CE0 = DW, DH  # reuse
    s.activation(LSE[:, :], S_[:, :], AF.Ln)
    tt(CE0[:, :], LSE[:, :], X0[:, :], OP.subtract, eng=g)
    P0, u_, FLN = DCY, bufs[0]["ovx"], DCX  # FLN must outlive sl1 scratch
    s.activation(P0[:, :], CE0[:, :], AF.Exp, scale=-1.0)
    ts(u_[:, :], P0[:, :], -1.0, OP.mult, 1.0, OP.add)
    tt(FLN[:, :], u_[:, :], u_[:, :], OP.mult, eng=g)
    stt(FLN[:, :], FLN[:, :], 0.1, u_[:, :], OP.mult, OP.mult)
    tt(FLN[:, :], FLN[:, :], CE0[:, :], OP.mult, eng=g)

    CEP, PP = bufs[1]["ovx"], bufs[1]["ovy"]
    tt(CEP[:, :], LSE[:, :], XL[:, :], OP.subtract, eng=g)
    s.activation(PP[:, :], CEP[:, :], AF.Exp, scale=-1.0)
    ts(u_[:, :], PP[:, :], -1.0, OP.mult, 1.0, OP.add)
    tt(PP[:, :], u_[:, :], u_[:, :], OP.mult, eng=g)
    stt(PP[:, :], PP[:, :], 0.25, CEP[:, :], OP.mult, OP.mult)
    YPs = bufs[0]["YP"]
    stt(YPs[:, :], PP[:, :], 1.0, POSM[:, :], OP.mult, OP.mult,
        accum=acc_cp[:, :])
    psum_scalar(acc_cp[:, :], SC[:, 6:7])  # cls_pos

    # ---------------- gt lookup + smooth-L1 + bisection (woven) ----------
    # SELACC/sl1 DVE ops fill the bisection's PE round-trip latency.
    AXYf = DX0[:, :]
    AWHf = DX1[:, :]
    AXY = AXYf.bitcast(I32)
    AWH = AWHf.bitcast(I32)
    v.memset(AXY, 0)
    v.memset(AWH, 0)
    tcfF = tcf[:, :]

    mid, d_s = nt([1, 1], I32), nt([1, 1], I32)
    sel_i, nsel_i = nt([1, 1], I32), nt([1, 1], I32)
    sel_f = nt([1, 1])
    tmp1 = nt([1, 1], I32)
    geM = SA  # reuse

    _bis = [0]
    bis_bc = [None]

    def bisect_step_a():  # mid + bcast launch (cheap tiny ops + PE)
        tt(d_s[:, :], hi[:, :], lo[:, :], OP.subtract)
        ts(d_s[:, :], d_s[:, :], 1, OP.arith_shift_right)
        tt(mid[:, :], lo[:, :], d_s[:, :], OP.add)
        bc = psum.tile([P, 1], F32, name="bcc", tag="bcc")
        nc.tensor.matmul(bc[:, :], ones_row[:, :], mid[:, :].bitcast(F32),
                         start=True, stop=True)
        return bc

    def bisect_step_b(bc):  # count + select + window update
        ts(geM[:, :], NEGY[:, :], bc[:, 0:1], OP.is_le, accum=acc_cnt[:, :])
        pt = psum.tile([1, 1], F32, name="psb", tag="psb")
        nc.tensor.matmul(pt[:, :], acc_cnt[:, :], ones_col[:, :],
                         start=True, stop=True)
        tt(sel_f[:, :], pt[:, :], k_t, OP.is_ge)
        ts(sel_i[:, :], sel_f[:, :], 1, OP.mult)
        ts(nsel_i[:, :], sel_i[:, :], -1, OP.mult, 1, OP.add)  # 1-sel
        tt(d_s[:, :], mid[:, :], hi[:, :], OP.subtract)
        tt(d_s[:, :], d_s[:, :], sel_i[:, :], OP.mult)
        tt(hi[:, :], hi[:, :], d_s[:, :], OP.add)
        tt(tmp1[:, :], mid[:, :], lo[:, :], OP.subtract)
        tt(tmp1[:, :], tmp1[:, :], nsel_i[:, :], OP.mult)
        tt(lo[:, :], lo[:, :], tmp1[:, :], OP.add)

    def weave():
        # one bisection half-step between chunks of SELACC/sl1 work
        if _bis[0] >= 2 * NBIS:
            return
        if _bis[0] % 2 == 0:
            bis_bc[0] = bisect_step_a()
        else:
            bisect_step_b(bis_bc[0])
        _bis[0] += 1

    for t in range(T):
        v._custom_dve(SELACC, out=AXYf, in0=tcfF, in1=AXYf,
                      s0=float(t), s1=QXY[:, t:t + 1].bitcast(F32))
        v._custom_dve(SELACC, out=AWHf, in0=tcfF, in1=AWHf,
                      s0=float(t), s1=QWH[:, t:t + 1].bitcast(F32))
        weave()

    # ---------------- unpack gt + smooth-L1 (pipelined over coords) -------
    Gx, Gy, Gw, Gh = (ANv[:, i, :] for i in range(4))  # reuse AN storage
    gq = xqi
    gq2 = nt([P, FD], I32)
    ts(gq[:, :], AXY, 16, OP.arith_shift_right)
    ts(Gx, gq[:, :], 0.03125, OP.mult)
    ts(gq2[:, :], AXY, 0xFFFF, OP.bitwise_and)
    ts(Gy, gq2[:, :], 0.03125, OP.mult)
    weave()
    GwR, GhR = nt([P, FD]), nt([P, FD])
    ts(gq[:, :], AWH, 16, OP.arith_shift_right)
    ts(GwR[:, :], gq[:, :], 0.000244140625, OP.mult)
    ts(gq2[:, :], AWH, 0xFFFF, OP.bitwise_and)
    ts(GhR[:, :], gq2[:, :], 0.000244140625, OP.mult)
    weave()

    SL = CE0  # dead after FLN chain
    v.memset(SL[:, :], 0.0)
    # four independent residual/sl1 pipelines (per-coord tiles)
    Rs = [P0, bufs[1]["Y"], bufs[1]["RI"], bufs[1]["I"]]
    sas = [bufs[0]["I"], bufs[0]["RI"], bufs[0]["Y"], bufs[0]["YP"]]
    sqs = [bufs[0]["U"], bufs[1]["U"], bufs[1]["YP"], gq.bitcast(F32)]
    slts = [bufs[0]["ovx"], bufs[0]["ovy"], bufs[1]["ovx"], bufs[1]["ovy"]]
    tt(Rs[0][:, :], Gx, RBX[:, :], OP.mult, eng=g)
    tt(Rs[1][:, :], Gy, RBY[:, :], OP.mult, eng=g)
    tt(Rs[0][:, :], ALX[:, :], Rs[0][:, :], OP.subtract)
    tt(Rs[1][:, :], ALY[:, :], Rs[1][:, :], OP.subtract)
    stt(Rs[2][:, :], GWl[:, :], 2.0, GwR[:, :], OP.add, OP.subtract)
    stt(Rs[3][:, :], GHl[:, :], 2.0, GhR[:, :], OP.add, OP.subtract)
    weave()
    for kk in range(4):  # abs + square stage
        ts(sas[kk][:, :].bitcast(I32), Rs[kk][:, :].bitcast(I32),
           0x7FFFFFFF, OP.bitwise_and)
        tt(sqs[kk][:, :], Rs[kk][:, :], Rs[kk][:, :], OP.mult, eng=g)
        weave()
    for kk in range(4):  # piecewise combine: lin + slt*(0.5*sq - lin)
        lin = Rs[kk]
        ts(lin[:, :], sas[kk][:, :], -0.5, OP.add)
        ts(slts[kk][:, :], sas[kk][:, :], 1.0, OP.is_lt)
        stt(sqs[kk][:, :], sqs[kk][:, :], 0.5, lin[:, :], OP.mult, OP.subtract)
        tt(sqs[kk][:, :], sqs[kk][:, :], slts[kk][:, :], OP.mult, eng=g)
        tt(sqs[kk][:, :], sqs[kk][:, :], lin[:, :], OP.add, eng=g)
        weave()
    tt(SL[:, :], sqs[0][:, :], sqs[1][:, :], OP.add, eng=g)
    tt(sqs[2][:, :], sqs[2][:, :], sqs[3][:, :], OP.add, eng=g)
    weave()
    tt(SL[:, :], SL[:, :], sqs[2][:, :], OP.add, eng=g)
    while _bis[0] < 2 * NBIS:
        weave()

    stt(bufs[1]["U"][:, :], SL[:, :], 1.0, POSM[:, :], OP.mult, OP.mult,
        accum=acc_sl[:, :])
    psum_scalar(acc_sl[:, :], SC[:, 7:8])  # sl1 sum
    geM2 = bufs[1]["ovy"]
    YPs2 = bufs[1]["ovx"]
    acc_cnt2, acc_sum2 = nt([P, 1]), nt([P, 1])
    bc_lo = bcast_col(lo[:, :].bitcast(F32))
    bc_hi = bcast_col(hi[:, :].bitcast(F32))
    ts(geM[:, :], NEGY[:, :], bc_lo[:, 0:1], OP.is_le, accum=acc_cnt[:, :])
    ts(geM2[:, :], NEGY[:, :], bc_hi[:, 0:1], OP.is_le, accum=acc_cnt2[:, :])
    psum_scalar(acc_cnt[:, :], SC[:, 2:3])
    psum_scalar(acc_cnt2[:, :], SC[:, 4:5])
    stt(YPs[:, :], FLN[:, :], 1.0, geM[:, :], OP.mult, OP.mult,
        accum=acc_sum[:, :])
    stt(YPs2[:, :], FLN[:, :], 1.0, geM2[:, :], OP.mult, OP.mult,
        accum=acc_sum2[:, :])
    psum_scalar(acc_sum[:, :], SC[:, 3:4])
    psum_scalar(acc_sum2[:, :], SC[:, 5:6])
    nc.sync.dma_start(out_d[:, :], SC[:, :])
    ctx.close()


def _make_in_maps(cls_output, reg_output, anchors, target_boxes, target_labels):
    B = cls_output.shape[0]
    import ml_dtypes
    bf16 = ml_dtypes.bfloat16
    in_maps = []
    tcode = np.arange(T, dtype=np.int32)[:, None, None]
    # anchor-minor layout: a = f*128 + p  ->  [p, f]
    anc_pf = np.ascontiguousarray(
        np.asarray(anchors, np.float32).reshape(FD, P, 4).swapaxes(0, 1)
        .reshape(P, 4 * FD))
    for b in range(B):
        cls_b = np.asarray(cls_output[b], dtype=np.float32)
        labels_b = np.asarray(target_labels[b]).astype(np.int64)
        # 7-bit quantized gathered logits + 5-bit target index, [T, P, FD]
        xl = cls_b[:, labels_b].T.reshape(T, FD, P).swapaxes(1, 2)
        xq = np.clip(np.floor(xl * 8.0 + 64.5), 0, 127).astype(np.int32)
        meta = (xq << 5) | tcode
        in_maps.append({
            "clsT": np.ascontiguousarray(cls_b.T).astype(bf16),
            "meta": np.ascontiguousarray(meta),
            "reg": np.ascontiguousarray(
                np.asarray(reg_output[b], np.float32).reshape(FD, P, 4)
                .swapaxes(0, 1).reshape(P, 4 * FD)),
            "anc": anc_pf,
            "x0": np.ascontiguousarray(cls_b[:, 0].reshape(FD, P).T),
            "tb": np.ascontiguousarray(target_boxes[b], dtype=np.float32),
        })
    return in_maps


def kernel(cls_output, reg_output, anchors, target_boxes, target_labels):
    global _compiled
    if _compiled is None:
        _compiled = _build()
    nc = _compiled
    B = cls_output.shape[0]
    in_maps = _make_in_maps(cls_output, reg_output, anchors, target_boxes,
                            target_labels)
    res = bass_utils.run_bass_kernel_spmd(nc, in_maps, core_ids=list(range(B)))

    cls_l = np.zeros(B, np.float32)
    reg_l = np.zeros(B, np.float32)
    npos_a = np.zeros(B, np.int64)
    for b in range(B):
        sc = res.results[b]["out"][0]
        npos, k = float(sc[0]), float(sc[1])
        c_lo, s_lo, c_hi, s_hi = float(sc[2]), float(sc[3]), float(sc[4]), float(sc[5])
        cls_pos, sl1s = float(sc[6]), float(sc[7])
        # fractional blend on the bisection plateau: exactly k negatives
        if c_hi > c_lo:
            frac = (k - c_lo) / (c_hi - c_lo)
        else:
            frac = 0.0
        cls_neg = s_lo + frac * (s_hi - s_lo)
        total = max(npos + k, 1.0)
        cls_l[b] = np.float32((cls_pos + cls_neg) / total)
        reg_l[b] = np.float32(sl1s / (npos + 1e-6))
        npos_a[b] = int(round(npos))

    total_pos = np.int32(npos_a.sum())
    cls_final = np.float32(cls_l.mean())
    reg_final = np.float32(reg_l.mean()) if total_pos > 0 else np.float32(0.0)
    reg_weight = np.float32(min(1.0, float(total_pos) / (100.0 * B)))
    total_loss = np.float32(cls_final + reg_weight * 1.0 * reg_final)
    return (total_loss, cls_final, reg_final, np.int32(total_pos))


# revision 25
# speedup vs baseline: 1.3385x; 1.0043x over previous
"""Trainium2 Bass kernel for nn_DetectionLoss (anchor matching + focal/smooth-L1).

Pure data parallelism: image b runs on core b (B=8). Each core emits 8 partial
scalars; the host combines them (same final reduction as the reference).

Device algorithm (N=65536 anchors, T=32 targets, C=80 classes), v2:
  - Work with the inverted score y = U/I (U = Sa+Sb+eps, I = intersection);
    y is strictly decreasing in IoU, so argmax-IoU = argmin-y and
    iou >= 0.5  <=>  y <= 3 (exactly the reference's threshold algebra).
  - Per t: y computed in ONE scalar_tensor_tensor op ((SA+sbe_t)/I); overlap
    widths via one fused custom DVE op each (relu(min-max)+1e-15, the epsilon
    keeps y finite so no NaN/inf enters the min-reduce); the packed min over t
    carries 12 bits of metadata (7-bit quantized matched-class logit + 5-bit
    target index) in the fp32 mantissa low bits; I-mult and min-accumulate run
    on the otherwise-idle GPSIMD engine.
  - Classification: cls arrives host-transposed as bf16 [C, N]; exp on the
    scalar engine, per-anchor softmax denominator S via PE matmul with a ones
    vector (contraction over the 80 class partitions), landed into anchor-major
    layout by tiny PSUM->SBUF DMAs. ce = log S - x; focal sums via
    tensor_scalar/stt accum_out fused row-sums + PE column-sum.
  - Hard negatives: rank by y ascending among non-positives; k-th threshold by
    bisection on the int32 bit pattern of y (positive floats are bit-ordered),
    host blends the boundary plateau fractionally (exact top-k to ~1e-4).
  - Matched-gt lookup for smooth-L1: 16+16-bit packed gt quantities selected
    per anchor by a custom select-or-accumulate op over the 5-bit target code
    (disjoint one-hot masks => bitwise-OR accumulate).

Host-side prep (sharding/layout only, no loss math): per-image transpose+cast
of cls to bf16 [C, N], gather of the 32 labeled logit columns quantized into
the 12-bit metadata words, and the final 8->1 scalar reduction/blend.
"""

import sys, os

for _p in ("/opt/trn_rl_repo",):
    if _p not in sys.path:
        sys.path.insert(0, _p)

import numpy as np

import concourse.bass as bass
import concourse.bacc as bacc
import concourse.mybir as mybir
from concourse.tile import TileContext
from concourse import bass_utils

F32 = mybir.dt.float32
BF16 = mybir.dt.bfloat16
I32 = mybir.dt.int32
OP = mybir.AluOpType
AF = mybir.ActivationFunctionType

N, C, T = 65536, 80, 32
P, FD = 128, 512  # anchor a = p*FD + f
NCORES = 8
NBIS = 12
CCH = 2048        # anchors per cls-stream chunk
NCH = N // CCH
Y_LO_BITS = 0x40400000   # bits(3.0)
Y_HI_BITS = 0x5D000000   # ~5.8e17, above any finite y (eps=1e-6 keeps y <~ 1.5e17)
POS_OFF = 1e35           # added to y of positives so they rank last

_compiled = None


def _register_dve_op(name, spec):
    from concourse import dve_ops as DOPS
    from concourse.dve_spec import lower
    from concourse.dve_table_gen import DveOpSpec
    if name in DOPS._SUB_OPCODE_FOR_NAME:
        return next(o for o in DOPS.OPS if o.name == name)
    DOPS.OPS.append(DOPS.DveOp(name, spec, False, {}))
    DOPS._SUB_OPCODE_FOR_NAME[name] = DOPS._CUSTOM_DVE_ROW_BASE + len(DOPS.OPS) - 1
    DOPS.CUSTOM_DVE_SPECS[name] = spec
    opc = DOPS.get_dve_sub_opcode(name)
    shas = {}
    for ver in ("v3", "v4"):
        shas[ver] = DveOpSpec(name=name, opcode=opc, uops=lower(spec, ver=ver),
                              rd1_en=DOPS.has_src1(spec)).sha(ver)
    DOPS.OPS[-1] = DOPS.DveOp(name, spec, False, shas)
    return DOPS.OPS[-1]


def _get_ops():
    import numpy as _np
    from concourse.dve_spec import (Spec, Src0, Src1, C0, C1, C2, Zero, relu,
                                    minn, maxx, select, eq, Bin, AluOp)

    def _bits(a):
        return _np.asarray(a, _np.float32).view(_np.int32)

    def _ovlpe_ref(in0, in1, s0, s1, imm2):
        return _np.maximum(_np.minimum(in0, s0) - _np.maximum(in1, s1), 0.0) + imm2

    def _pack_ref(in0, in1, s0, s1):
        m = _bits(s0)
        return ((_bits(in0) & ~m) | _bits(in1)).view(_np.float32)

    def _selacc_ref(in0, in1, s0, s1):
        pick = _np.where(_np.asarray(in0, _np.float32) == _np.float32(s0),
                         _np.broadcast_to(_bits(s1), in0.shape), 0)
        return (_bits(in1) | pick).view(_np.float32)

    # overlap width + tiny epsilon: relu(min(Src0,C0) - max(Src1,C1)) + C2
    ovlpe = _register_dve_op(
        "ANT_DL_OVLPE",
        Spec(body=Bin(AluOp.ADD, relu(minn(Src0, C0) - maxx(Src1, C1)), C2),
             reference=lambda in0, in1, s0, s1, imm2: _ovlpe_ref(in0, in1, s0, s1, imm2)))
    # clear low-12 bits of Src0, OR in Src1 (metadata)
    pack = _register_dve_op(
        "ANT_DL_PACK",
        Spec(body=Bin(AluOp.BITWISE_OR,
                      Bin(AluOp.BITWISE_XOR, Src0,
                          Bin(AluOp.BITWISE_AND, Src0, C0)), Src1),
             reference=lambda in0, in1, s0, s1, imm2: _pack_ref(in0, in1, s0, s1)))
    # select-or-accumulate: Src1 | (Src0 == C0 ? C1 : 0)   (disjoint masks)
    selacc = _register_dve_op(
        "ANT_DL_SELACC",
        Spec(body=Bin(AluOp.BITWISE_OR, Src1, select(eq(Src0, C0), C1, Zero)),
             reference=lambda in0, in1, s0, s1, imm2: _selacc_ref(in0, in1, s0, s1)))
    return ovlpe, pack, selacc


def _build():
    nc = bacc.Bacc("TRN2", target_bir_lowering=False, debug=False,
                   enable_asserts=False, num_devices=NCORES)
    clsT_d = nc.dram_tensor("clsT", [C, N], BF16, kind="ExternalInput")
    meta_d = nc.dram_tensor("meta", [T, P, FD], I32, kind="ExternalInput")
    reg_d = nc.dram_tensor("reg", [P, 4 * FD], F32, kind="ExternalInput")
    anc_d = nc.dram_tensor("anc", [P, 4 * FD], F32, kind="ExternalInput")
    x0_d = nc.dram_tensor("x0", [P, FD], F32, kind="ExternalInput")
    tb_d = nc.dram_tensor("tb", [T, 4], F32, kind="ExternalInput")
    out_d = nc.dram_tensor("out", [1, 16], F32, kind="ExternalOutput")

    with TileContext(nc) as tc:
        _emit(nc, tc, clsT_d, meta_d, reg_d, anc_d, x0_d, tb_d, out_d)
    nc.compile()
    return nc


def _emit(nc, tc, clsT_d, meta_d, reg_d, anc_d, x0_d, tb_d, out_d):
    import contextlib
    ctx = contextlib.ExitStack()
    pool = ctx.enter_context(tc.tile_pool(name="main", bufs=1))
    cpool = ctx.enter_context(tc.tile_pool(name="cls", bufs=2))
    mpool = ctx.enter_context(tc.tile_pool(name="meta", bufs=2))
    psum = ctx.enter_context(tc.tile_pool(name="ps", bufs=1, space="PSUM"))
    pssum = ctx.enter_context(tc.tile_pool(name="psS", bufs=2, space="PSUM"))
    v, s, g = nc.vector, nc.scalar, nc.gpsimd

    def ts(out, in0, s1, op0, s2=None, op1=None, accum=None, eng=v):
        if accum is not None and op1 is None:
            op1 = OP.add  # accum reduce op rides in op1
        kw = dict(scalar2=s2) if op1 is None else dict(scalar2=s2, op1=op1)
        if accum is not None:
            kw["accum_out"] = accum
        return eng.tensor_scalar(out=out, in0=in0, scalar1=s1, op0=op0, **kw)

    def tt(out, in0, in1, op, eng=v):
        return eng.tensor_tensor(out=out, in0=in0, in1=in1, op=op)

    def stt(out, in0, sc, in1, op0, op1, accum=None, eng=v):
        kw = {} if accum is None else {"accum_out": accum}
        return eng.scalar_tensor_tensor(out=out, in0=in0, scalar=sc, in1=in1,
                                        op0=op0, op1=op1, **kw)

    _ctr = [0]

    def nt(shape, dt=F32):
        _ctr[0] += 1
        return pool.tile(shape, dt, name=f"tl{_ctr[0]}", tag=f"tl{_ctr[0]}")

    OVLPE, PACKOP, SELACC = _get_ops()

    # ---------------- input DMAs (priority order: t-loop deps first) -------
    TB = nt([P, 4 * T])
    nc.sync.dma_start(TB[:, :], tb_d.rearrange("t c -> (t c)")[None, :]
                      .broadcast_to([P, 4 * T]))
    RG, AN = nt([P, 4 * FD]), nt([P, 4 * FD])
    nc.sync.dma_start(AN[:, :], anc_d[:, :])
    nc.sync.dma_start(RG[:, :], reg_d[:, :])
    X0 = nt([P, FD])
    nc.sync.dma_start(X0[:, :], x0_d[:, :])

    # ---------------- target prep (tiny) ----------------
    TBv = TB[:, :].rearrange("p (t c) -> p t c", c=4)
    tx0, ty0 = TBv[:, :, 0], TBv[:, :, 1]
    tx1, ty1 = TBv[:, :, 2], TBv[:, :, 3]
    WB, HB, SBE = nt([P, T]), nt([P, T]), nt([P, T])
    GCX, GCY, LNW, LNH = nt([P, T]), nt([P, T]), nt([P, T]), nt([P, T])
    tt(WB[:, :], tx1, tx0, OP.subtract)
    tt(HB[:, :], ty1, ty0, OP.subtract)
    tmpT = nt([P, T])
    tt(tmpT[:, :], WB[:, :], HB[:, :], OP.mult)
    ts(SBE[:, :], tmpT[:, :], 1e-6, OP.add)
    stt(GCX[:, :], WB[:, :], 0.5, tx0, OP.mult, OP.add)
    stt(GCY[:, :], HB[:, :], 0.5, ty0, OP.mult, OP.add)
    s.activation(LNW[:, :], WB[:, :], AF.Ln)
    s.activation(LNH[:, :], HB[:, :], AF.Ln)
    # 16+16-bit packed gt coords (x<<16|y), (lnw<<16|lnh)
    QXY, QWH = nt([P, T], I32), nt([P, T], I32)
    q0, q1 = nt([P, T], I32), nt([P, T], I32)
    ts(q0[:, :], GCX[:, :], 32.0, OP.mult)
    ts(q1[:, :], GCY[:, :], 32.0, OP.mult)
    qt = nt([P, T], I32)
    ts(qt[:, :], q0[:, :], 65536, OP.mult)
    tt(QXY[:, :], qt[:, :], q1[:, :], OP.add)
    ts(q0[:, :], LNW[:, :], 4096.0, OP.mult, 8192.0, OP.add)
    ts(q1[:, :], LNH[:, :], 4096.0, OP.mult, 8192.0, OP.add)
    ts(qt[:, :], q0[:, :], 65536, OP.mult)
    tt(QWH[:, :], qt[:, :], q1[:, :], OP.add)

    # constants / accumulators early (fills the input-DMA wait)
    SC = nt([1, 16])
    v.memset(SC[:, :], 0.0)
    ones_col = nt([P, 1])
    v.memset(ones_col[:, :], 1.0)
    ones_row = nt([1, P])
    v.memset(ones_row[:, :], 1.0)
    acc_np, acc_cnt, acc_sum, acc_cp, acc_sl = (nt([P, 1]) for _ in range(5))
    MACC = nt([P, FD])
    v.memset(MACC[:, :].bitcast(I32), Y_HI_BITS + 0x3FFFFF)
    MSKC = nt([P, 1], I32)
    v.memset(MSKC[:, :], 0xFFF)
    lo, hi = nt([1, 1], I32), nt([1, 1], I32)
    v.memset(lo[:, :], Y_LO_BITS)
    v.memset(hi[:, :], Y_HI_BITS)

    # ---------------- anchor prep ----------------
    RGv = RG[:, :].rearrange("p (f c) -> p c f", c=4)
    ANv = AN[:, :].rearrange("p (f c) -> p c f", c=4)
    rg0, rg1, rg2, rg3 = (RGv[:, i, :] for i in range(4))
    ax0, ay0, ax1, ay1 = (ANv[:, i, :] for i in range(4))

    AW, AH, ACX, ACY = nt([P, FD]), nt([P, FD]), nt([P, FD]), nt([P, FD])
    tt(AW[:, :], ax1, ax0, OP.subtract)
    tt(AH[:, :], ay1, ay0, OP.subtract)
    stt(ACX[:, :], AW[:, :], 0.5, ax0, OP.mult, OP.add)
    stt(ACY[:, :], AH[:, :], 0.5, ay0, OP.mult, OP.add)

    # anc-only reg-target helpers first (reg DMA still in flight)
    AWE, AHE = nt([P, FD]), nt([P, FD])
    ts(AWE[:, :], AW[:, :], 1e-6, OP.add)
    ts(AHE[:, :], AH[:, :], 1e-6, OP.add)
    RBX, RBY, rsc = nt([P, FD]), nt([P, FD]), nt([P, FD])
    v.reciprocal_approx_accurate(out=RBX[:, :], in_=AWE[:, :], scratch=rsc[:, :])
    v.reciprocal_approx_accurate(out=RBY[:, :], in_=AHE[:, :], scratch=rsc[:, :])

    DW, DH = nt([P, FD]), nt([P, FD])
    s.activation(DW[:, :], rg2, AF.Exp)
    s.activation(DH[:, :], rg3, AF.Exp)
    tt(DW[:, :], DW[:, :], AW[:, :], OP.mult, eng=g)
    tt(DH[:, :], DH[:, :], AH[:, :], OP.mult, eng=g)
    DCX, DCY = nt([P, FD]), nt([P, FD])
    tt(DCX[:, :], AW[:, :], rg0, OP.mult, eng=g)
    tt(DCX[:, :], DCX[:, :], ACX[:, :], OP.add)
    tt(DCY[:, :], AH[:, :], rg1, OP.mult, eng=g)
    tt(DCY[:, :], DCY[:, :], ACY[:, :], OP.add)
    DX0, DY0, DX1, DY1, SA = (nt([P, FD]) for _ in range(5))
    stt(DX0[:, :], DW[:, :], -0.5, DCX[:, :], OP.mult, OP.add)
    stt(DX1[:, :], DW[:, :], 0.5, DCX[:, :], OP.mult, OP.add)
    stt(DY0[:, :], DH[:, :], -0.5, DCY[:, :], OP.mult, OP.add)
    stt(DY1[:, :], DH[:, :], 0.5, DCY[:, :], OP.mult, OP.add)
    tt(SA[:, :], DW[:, :], DH[:, :], OP.mult)
    ALX, ALY, GWl, GHl = nt([P, FD]), nt([P, FD]), nt([P, FD]), nt([P, FD])
    s.activation(GWl[:, :], AWE[:, :], AF.Ln)
    s.activation(GHl[:, :], AHE[:, :], AF.Ln)
    prep_late = [
        lambda: tt(ALX[:, :], ACX[:, :], RBX[:, :], OP.mult, eng=g),
        lambda: tt(ALX[:, :], ALX[:, :], rg0, OP.add, eng=g),
        lambda: tt(ALY[:, :], ACY[:, :], RBY[:, :], OP.mult, eng=g),
        lambda: tt(ALY[:, :], ALY[:, :], rg1, OP.add, eng=g),
        lambda: tt(GWl[:, :], GWl[:, :], rg2, OP.add, eng=g),
        lambda: tt(GHl[:, :], GHl[:, :], rg3, OP.add, eng=g),
    ]

    # ---------------- cls stream pieces (embedded in t-loop) --------------
    S_ = nt([P, FD])
    ones80 = nt([C, 1], BF16)
    v.memset(ones80[:, :], 1.0)
    MCH = CCH // P  # S columns per chunk

    def emit_cls_chunk(ch):
        CT = cpool.tile([C, CCH], BF16, name="ct", tag="ct")
        nc.sync.dma_start(CT[:, :], clsT_d[:, ch * CCH:(ch + 1) * CCH])
        s.activation(CT[:, :], CT[:, :], AF.Exp)
        pS = pssum.tile([P, MCH], F32, name="pS", tag="pS")
        for j in range(MCH):
            nc.tensor.matmul(pS[:, j:j + 1], CT[:, j * P:(j + 1) * P],
                             ones80[:, :], start=True, stop=True)
        s.activation(S_[:, ch * MCH:(ch + 1) * MCH], pS[:, :], AF.Copy)

    # ---------------- t-loop: packed min over targets (sw-pipelined) ------
    TPC = 4  # meta rows per streamed chunk
    NB = 3
    bufs = [dict(ovx=nt([P, FD]), ovy=nt([P, FD]), I=nt([P, FD]),
                 RI=nt([P, FD]), Y=nt([P, FD]), YP=nt([P, FD]),
                 U=nt([P, FD])) for _ in range(NB)]
    mtv = [None, None]

    def fetch_meta(c, eng=None):
        if c >= T // TPC:
            return
        MT = mpool.tile([P, TPC * FD], I32, name="mt", tag="mt")
        (eng or s).dma_start(MT[:, :].rearrange("p (t f) -> p t f", f=FD),
                             meta_d[c * TPC:(c + 1) * TPC, :, :]
                             .rearrange("t p f -> p t f"))
        mtv[c % 2] = MT[:, :].rearrange("p (t f) -> p t f", f=FD)

    def emit_front(t):
        b = bufs[t % NB]
        if t % TPC == 1:
            fetch_meta((t - 1) // TPC + 1)  # prefetch next chunk
        v._custom_dve(OVLPE, out=b["ovx"][:, :], in0=DX1[:, :], in1=DX0[:, :],
                      s0=TB[:, 4 * t + 2:4 * t + 3],
                      s1=TB[:, 4 * t + 0:4 * t + 1], imm2=1e-6)
        v._custom_dve(OVLPE, out=b["ovy"][:, :], in0=DY1[:, :], in1=DY0[:, :],
                      s0=TB[:, 4 * t + 3:4 * t + 4],
                      s1=TB[:, 4 * t + 1:4 * t + 2], imm2=1e-6)
        s.activation(b["U"][:, :], SA[:, :], AF.Identity, bias=SBE[:, t:t + 1])
        tt(b["I"][:, :], b["ovx"][:, :], b["ovy"][:, :], OP.mult, eng=g)

    def emit_back(t):
        b = bufs[t % NB]
        v.reciprocal_approx_fast(out=b["RI"][:, :], in_=b["I"][:, :])
        tt(b["Y"][:, :], b["U"][:, :], b["RI"][:, :], OP.mult, eng=g)
        v._custom_dve(PACKOP, out=b["YP"][:, :], in0=b["Y"][:, :],
                      in1=mtv[(t // TPC) % 2][:, t % TPC, :].bitcast(F32),
                      s0=MSKC[:, :].bitcast(F32))
        tt(MACC[:, :], MACC[:, :], b["YP"][:, :], OP.min)

    fetch_meta(0, eng=nc.sync)
    emit_front(0)
    for t in range(1, T):
        emit_front(t)
        emit_cls_chunk(t - 1)
        if prep_late:
            prep_late.pop(0)()
        emit_back(t - 1)
    emit_cls_chunk(31)
    emit_back(T - 1)

    # ---------------- selection scalars ----------------
    def psum_scalar(src_col, dst):  # [P,1] -> [1,1]
        pt = psum.tile([1, 1], F32, name="pss", tag="pss")
        nc.tensor.matmul(pt[:, :], src_col, ones_col[:, :], start=True, stop=True)
        ts(dst, pt[:, :], 1.0, OP.mult)

    def bcast_col(src_sc):  # [1,1] -> [P,1]
        bc = psum.tile([P, 1], F32, name="bcc", tag="bcc")
        nc.tensor.matmul(bc[:, :], ones_row[:, :], src_sc, start=True, stop=True)
        bcs = nt([P, 1])
        s.activation(bcs[:, :], bc[:, :], AF.Copy)
        return bcs

    mi = MACC[:, :].bitcast(I32)
    POSM, NEGY = AW, AH  # reuse dead prep tiles
    ts(POSM[:, :], MACC[:, :], 3.0, OP.is_le, accum=acc_np[:, :])
    npos_t = SC[:, 0:1]
    psum_scalar(acc_np[:, :], npos_t)
    stt(NEGY[:, :], POSM[:, :], POS_OFF, MACC[:, :], OP.mult, OP.add)

    # k = min(4*npos, N-npos)
    k_t = SC[:, 1:2]
    kA, kB = nt([1, 1]), nt([1, 1])
    ts(kA[:, :], npos_t, 4.0, OP.mult)
    ts(kB[:, :], npos_t, -1.0, OP.mult, float(N), OP.add)
    tt(k_t, kA[:, :], kB[:, :], OP.min)

    # decode metadata from MACC
    tcf, XL = ACX, ACY  # reuse
    xqi = nt([P, FD], I32)
    ts(xqi[:, :], mi, 31, OP.bitwise_and)
    ts(tcf[:, :], xqi[:, :], 1.0, OP.mult)          # tcode as f32
    ts(xqi[:, :], mi, 5, OP.arith_shift_right, 0x7F, OP.bitwise_and)
    ts(XL[:, :], xqi[:, :], 0.125, OP.mult, -8.0625, OP.add)

    # ---------------- fl_neg (dense) + cls_pos ----------------
    LSE, CE0 = DW, DH  # reuse
    s.activation(LSE[:, :], S_[:, :], AF.Ln)
    tt(CE0[:, :], LSE[:, :], X0[:, :], OP.subtract, eng=g)
    P0, u_, FLN = DCY, bufs[0]["ovx"], DCX  # FLN must outlive sl1 scratch
    s.activation(P0[:, :], CE0[:, :], AF.Exp, scale=-1.0)
    ts(u_[:, :], P0[:, :], -1.0, OP.mult, 1.0, OP.add)
    tt(FLN[:, :], u_[:, :], u_[:, :], OP.mult, eng=g)
    stt(FLN[:, :], FLN[:, :], 0.1, u_[:, :], OP.mult, OP.mult)
    tt(FLN[:, :], FLN[:, :], CE0[:, :], OP.mult, eng=g)

    CEP, PP = bufs[1]["ovx"], bufs[1]["ovy"]
    tt(CEP[:, :], LSE[:, :], XL[:, :], OP.subtract, eng=g)
    s.activation(PP[:, :], CEP[:, :], AF.Exp, scale=-1.0)
    ts(u_[:, :], PP[:, :], -1.0, OP.mult, 1.0, OP.add)
    tt(PP[:, :], u_[:, :], u_[:, :], OP.mult, eng=g)
    stt(PP[:, :], PP[:, :], 0.25, CEP[:, :], OP.mult, OP.mult)
    YPs = bufs[0]["YP"]
    stt(YPs[:, :], PP[:, :], 1.0, POSM[:, :], OP.mult, OP.mult,
        accum=acc_cp[:, :])
    psum_scalar(acc_cp[:, :], SC[:, 6:7])  # cls_pos

    # ---------------- gt lookup + smooth-L1 + bisection (woven) ----------
    # SELACC/sl1 DVE ops fill the bisection's PE round-trip latency.
    AXYf = DX0[:, :]
    AWHf = DX1[:, :]
    AXY = AXYf.bitcast(I32)
    AWH = AWHf.bitcast(I32)
    v.memset(AXY, 0)
    v.memset(AWH, 0)
    tcfF = tcf[:, :]

    mid, d_s = nt([1, 1], I32), nt([1, 1], I32)
    sel_i, nsel_i = nt([1, 1], I32), nt([1, 1], I32)
    sel_f = nt([1, 1])
    tmp1 = nt([1, 1], I32)
    geM = SA  # reuse

    _bis = [0]
    bis_bc = [None]

    def bisect_step_a():  # mid + bcast launch (cheap tiny ops + PE)
        tt(d_s[:, :], hi[:, :], lo[:, :], OP.subtract)
        ts(d_s[:, :], d_s[:, :], 1, OP.arith_shift_right)
        tt(mid[:, :], lo[:, :], d_s[:, :], OP.add)
        bc = psum.tile([P, 1], F32, name="bcc", tag="bcc")
        nc.tensor.matmul(bc[:, :], ones_row[:, :], mid[:, :].bitcast(F32),
                         start=True, stop=True)
        return bc

    def bisect_step_b(bc):  # count + select + window update
        ts(geM[:, :], NEGY[:, :], bc[:, 0:1], OP.is_le, accum=acc_cnt[:, :])
        pt = psum.tile([1, 1], F32, name="psb", tag="psb")
        nc.tensor.matmul(pt[:, :], acc_cnt[:, :], ones_col[:, :],
                         start=True, stop=True)
        tt(sel_f[:, :], pt[:, :], k_t, OP.is_ge)
        ts(sel_i[:, :], sel_f[:, :], 1, OP.mult)
        ts(nsel_i[:, :], sel_i[:, :], -1, OP.mult, 1, OP.add)  # 1-sel
        tt(d_s[:, :], mid[:, :], hi[:, :], OP.subtract)
        tt(d_s[:, :], d_s[:, :], sel_i[:, :], OP.mult)
        tt(hi[:, :], hi[:, :], d_s[:, :], OP.add)
        tt(tmp1[:, :], mid[:, :], lo[:, :], OP.subtract)
        tt(tmp1[:, :], tmp1[:, :], nsel_i[:, :], OP.mult)
        tt(lo[:, :], lo[:, :], tmp1[:, :], OP.add)

    def weave():
        # one bisection half-step between chunks of SELACC/sl1 work
        if _bis[0] >= 2 * NBIS:
            return
        if _bis[0] % 2 == 0:
            bis_bc[0] = bisect_step_a()
        else:
            bisect_step_b(bis_bc[0])
        _bis[0] += 1

    for t in range(T):
        v._custom_dve(SELACC, out=AXYf, in0=tcfF, in1=AXYf,
                      s0=float(t), s1=QXY[:, t:t + 1].bitcast(F32))
        v._custom_dve(SELACC, out=AWHf, in0=tcfF, in1=AWHf,
                      s0=float(t), s1=QWH[:, t:t + 1].bitcast(F32))
        weave()

    # ---------------- unpack gt + smooth-L1 (pipelined over coords) -------
    Gx, Gy, Gw, Gh = (ANv[:, i, :] for i in range(4))  # reuse AN storage
    gq = xqi
    gq2 = nt([P, FD], I32)
    ts(gq[:, :], AXY, 16, OP.arith_shift_right)
    ts(Gx, gq[:, :], 0.03125, OP.mult)
    ts(gq2[:, :], AXY, 0xFFFF, OP.bitwise_and)
    ts(Gy, gq2[:, :], 0.03125, OP.mult)
    weave()
    GwR, GhR = nt([P, FD]), nt([P, FD])
    ts(gq[:, :], AWH, 16, OP.arith_shift_right)
    ts(GwR[:, :], gq[:, :], 0.000244140625, OP.mult)
    ts(gq2[:, :], AWH, 0xFFFF, OP.bitwise_and)
    ts(GhR[:, :], gq2[:, :], 0.000244140625, OP.mult)
    weave()

    SL = CE0  # dead after FLN chain
    v.memset(SL[:, :], 0.0)
    # four independent residual/sl1 pipelines (per-coord tiles)
    Rs = [P0, bufs[1]["Y"], bufs[1]["RI"], bufs[1]["I"]]
    sas = [bufs[0]["I"], bufs[0]["RI"], bufs[0]["Y"], bufs[0]["YP"]]
    sqs = [bufs[0]["U"], bufs[1]["U"], bufs[1]["YP"], gq.bitcast(F32)]
    slts = [bufs[0]["ovx"], bufs[0]["ovy"], bufs[1]["ovx"], bufs[1]["ovy"]]
    tt(Rs[0][:, :], Gx, RBX[:, :], OP.mult, eng=g)
    tt(Rs[1][:, :], Gy, RBY[:, :], OP.mult, eng=g)
    tt(Rs[0][:, :], ALX[:, :], Rs[0][:, :], OP.subtract)
    tt(Rs[1][:, :], ALY[:, :], Rs[1][:, :], OP.subtract)
    stt(Rs[2][:, :], GWl[:, :], 2.0, GwR[:, :], OP.add, OP.subtract)
    stt(Rs[3][:, :], GHl[:, :], 2.0, GhR[:, :], OP.add, OP.subtract)
    weave()
    for kk in range(4):  # abs + square stage
        ts(sas[kk][:, :].bitcast(I32), Rs[kk][:, :].bitcast(I32),
           0x7FFFFFFF, OP.bitwise_and)
        tt(sqs[kk][:, :], Rs[kk][:, :], Rs[kk][:, :], OP.mult, eng=g)
        weave()
    for kk in range(4):  # piecewise combine: lin + slt*(0.5*sq - lin)
        lin = Rs[kk]
        ts(lin[:, :], sas[kk][:, :], -0.5, OP.add)
        ts(slts[kk][:, :], sas[kk][:, :], 1.0, OP.is_lt)
        stt(sqs[kk][:, :], sqs[kk][:, :], 0.5, lin[:, :], OP.mult, OP.subtract)
        tt(sqs[kk][:, :], sqs[kk][:, :], slts[kk][:, :], OP.mult, eng=g)
        tt(sqs[kk][:, :], sqs[kk][:, :], lin[:, :], OP.add, eng=g)
        weave()
    tt(SL[:, :], sqs[0][:, :], sqs[1][:, :], OP.add, eng=g)
    tt(sqs[2][:, :], sqs[2][:, :], sqs[3][:, :], OP.add, eng=g)
    weave()
    tt(SL[:, :], SL[:, :], sqs[2][:, :], OP.add, eng=g)
    while _bis[0] < 2 * NBIS:
        weave()

    stt(bufs[1]["U"][:, :], SL[:, :], 1.0, POSM[:, :], OP.mult, OP.mult,
        accum=acc_sl[:, :])
    psum_scalar(acc_sl[:, :], SC[:, 7:8])  # sl1 sum
    geM2 = bufs[1]["ovy"]
    YPs2 = bufs[1]["ovx"]
    acc_cnt2, acc_sum2 = nt([P, 1]), nt([P, 1])
    bc_lo = bcast_col(lo[:, :].bitcast(F32))
    bc_hi = bcast_col(hi[:, :].bitcast(F32))
    ts(geM[:, :], NEGY[:, :], bc_lo[:, 0:1], OP.is_le, accum=acc_cnt[:, :])
    ts(geM2[:, :], NEGY[:, :], bc_hi[:, 0:1], OP.is_le, accum=acc_cnt2[:, :])
    psum_scalar(acc_cnt[:, :], SC[:, 2:3])
    psum_scalar(acc_cnt2[:, :], SC[:, 4:5])
    stt(YPs[:, :], FLN[:, :], 1.0, geM[:, :], OP.mult, OP.mult,
        accum=acc_sum[:, :])
    stt(YPs2[:, :], FLN[:, :], 1.0, geM2[:, :], OP.mult, OP.mult,
        accum=acc_sum2[:, :])
    psum_scalar(acc_sum[:, :], SC[:, 3:4])
    psum_scalar(acc_sum2[:, :], SC[:, 5:6])
    nc.sync.dma_start(out_d[:, :], SC[:, :])
    ctx.close()


def _make_in_maps(cls_output, reg_output, anchors, target_boxes, target_labels):
    B = cls_output.shape[0]
    import ml_dtypes
    bf16 = ml_dtypes.bfloat16
    in_maps = []
    tcode = np.arange(T, dtype=np.int32)[:, None, None]
    # anchor-minor layout: a = f*128 + p  ->  [p, f]
    anc_pf = np.ascontiguousarray(
        np.asarray(anchors, np.float32).reshape(FD, P, 4).swapaxes(0, 1)
        .reshape(P, 4 * FD))
    for b in range(B):
        cls_b = np.asarray(cls_output[b], dtype=np.float32)
        labels_b = np.asarray(target_labels[b]).astype(np.int64)
        # 7-bit quantized gathered logits + 5-bit target index, [T, P, FD]
        xl = cls_b[:, labels_b].T.reshape(T, FD, P).swapaxes(1, 2)
        xq = np.clip(np.floor(xl * 8.0 + 64.5), 0, 127).astype(np.int32)
        meta = (xq << 5) | tcode
        in_maps.append({
            "clsT": np.ascontiguousarray(cls_b.T).astype(bf16),
            "meta": np.ascontiguousarray(meta),
            "reg": np.ascontiguousarray(
                np.asarray(reg_output[b], np.float32).reshape(FD, P, 4)
                .swapaxes(0, 1).reshape(P, 4 * FD)),
            "anc": anc_pf,
            "x0": np.ascontiguousarray(cls_b[:, 0].reshape(FD, P).T),
            "tb": np.ascontiguousarray(target_boxes[b], dtype=np.float32),
        })
    return in_maps


def kernel(cls_output, reg_output, anchors, target_boxes, target_labels):
    global _compiled
    if _compiled is None:
        _compiled = _build()
    nc = _compiled
    B = cls_output.shape[0]
    in_maps = _make_in_maps(cls_output, reg_output, anchors, target_boxes,
                            target_labels)
    res = bass_utils.run_bass_kernel_spmd(nc, in_maps, core_ids=list(range(B)))

    cls_l = np.zeros(B, np.float32)
    reg_l = np.zeros(B, np.float32)
    npos_a = np.zeros(B, np.int64)
    for b in range(B):
        sc = res.results[b]["out"][0]
        npos, k = float(sc[0]), float(sc[1])
        c_lo, s_lo, c_hi, s_hi = float(sc[2]), float(sc[3]), float(sc[4]), float(sc[5])
        cls_pos, sl1s = float(sc[6]), float(sc[7])
        # fractional blend on the bisection plateau: exactly k negatives
        if c_hi > c_lo:
            frac = (k - c_lo) / (c_hi - c_lo)
        else:
            frac = 0.0
        cls_neg = s_lo + frac * (s_hi - s_lo)
        total = max(npos + k, 1.0)
        cls_l[b] = np.float32((cls_pos + cls_neg) / total)
        reg_l[b] = np.float32(sl1s / (npos + 1e-6))
        npos_a[b] = int(round(npos))

    total_pos = np.int32(npos_a.sum())
    cls_final = np.float32(cls_l.mean())
    reg_final = np.float32(reg_l.mean()) if total_pos > 0 else np.float32(0.0)
    reg_weight = np.float32(min(1.0, float(total_pos) / (100.0 * B)))
    total_loss = np.float32(cls_final + reg_weight * 1.0 * reg_final)
    return (total_loss, cls_final, reg_final, np.int32(total_pos))
